# revision 8
# baseline (speedup 1.0000x reference)
# Trainium2 Bass kernel for nn_DoseOptimizationLoss.
#
# Math: radiation(v) = sum_s exp(-(v-c_s)^T A_s (v-c_s)) over S=32 seeds on a
# 160^3 voxel grid. Loss needs only 4 big sums of functions of radiation.
#
# Key structure: for a fixed voxel row (i,j), quad(k) is a 1-D quadratic
#   quad(k) = C0(i,j) + Q(i,j)*k + alpha*k^2
# so each (128-row x 160-k) tile of quad for one seed is a rank-3 product ->
# computed on the tensor engine as part of a K=21 fp16 matmul covering 3 seeds
# (7 fp16 hi/lo-split rows per seed for fp32-grade precision), N=512 columns
# (3 seeds x 160 k + 32 pad cols that evaluate to quad=+big -> exp=0).
# ScalarE then does one wide Exp over each 2048-col PSUM fill, VectorE reduces
# the per-seed chunks into rad, and a short masked-sigmoid epilogue produces
# per-core partial sums which the host combines into the scalar loss.
#
# Sharding: voxel rows (flattened (i,j) axis, 25600 rows) split evenly across
# 8 cores; every core processes all 32 seeds over its 3200-row shard.

import numpy as np

import concourse.bass as bass
import concourse.bacc as bacc
import concourse.mybir as mybir
import concourse.tile as tile
from contextlib import ExitStack

VOL = 160
S = 32
SIGMA = np.array([8.0, 4.0, 4.0])
N_CORES = 8
ROWS = VOL * VOL            # 25600 (i,j) rows
RPC = ROWS // N_CORES       # 3200 rows per core
TILES = RPC // 128          # 25 tiles of 128 rows
NG = 11                     # seed groups: 10 triples + 1 pair
KT = 21                     # matmul contraction rows (7 per seed slot)
NCOL = 512                  # moving cols per matmul = one PSUM bank
PAD_BIG = 60000.0           # pad-col rhs value -> quad >= ~468 -> exp -> 0
ROUND_GROUPS = [(0, 1, 2, 3), (4, 5, 6, 7), (8, 9, 10)]

DVH_RATE = 0.9
F32 = mybir.dt.float32
F16 = mybir.dt.float16


def _seed_params(x):
    """float64 port of the reference's seed math: centers [S,3], A [S,3,3]."""
    xs = np.asarray(x, dtype=np.float64).reshape(S, 6)
    centers = xs[:, :3] * VOL
    d = xs[:, 3:]
    dot = d[:, 0]
    dot_c = np.clip(dot, -0.999999, 0.999999)
    angle = np.arccos(dot_c)
    z = np.zeros(S)
    axis_raw = np.stack([z, -d[:, 2], d[:, 1]], -1)
    nrm = np.linalg.norm(axis_raw, axis=-1, keepdims=True)
    axis = axis_raw / np.where(nrm > 1e-8, nrm, 1.0)
    cos_t = np.cos(angle)[:, None, None]
    sin_t = np.sin(angle)[:, None, None]
    a0, a1, a2 = axis[:, 0], axis[:, 1], axis[:, 2]
    K = np.stack([np.stack([z, -a2, a1], -1),
                  np.stack([a2, z, -a0], -1),
                  np.stack([-a1, a0, z], -1)], 1)
    eye = np.eye(3)
    R = cos_t * eye + (1.0 - cos_t) * axis[:, :, None] * axis[:, None, :] + sin_t * K
    R = np.where((np.abs(dot) >= 0.99)[:, None, None], eye, R)
    D = np.diag(1.0 / (2.0 * SIGMA ** 2))
    A = np.einsum('ski,kl,slj->sij', R, D, R)
    return centers, A


def _split16(v):
    hi = np.asarray(v, np.float64).astype(np.float16)
    lo = (np.asarray(v, np.float64) - hi.astype(np.float64)).astype(np.float16)
    return hi, lo


def _rhs_table():
    """k-pattern matrix [KT, NG*NCOL] fp16, identical for every core/tile."""
    k = np.arange(VOL, dtype=np.float64)
    k2hi, k2lo = _split16(k * k)
    rhs = np.zeros((KT, NG * NCOL), np.float16)
    for g in range(NG):
        base = g * NCOL
        nslots = 3 if g < 10 else 2
        for sl in range(nslots):
            r = 7 * sl
            c = base + 160 * sl
            rhs[r + 0, c:c + 160] = np.float16(1.0)
            rhs[r + 1, c:c + 160] = np.float16(1.0)
            rhs[r + 2, c:c + 160] = k.astype(np.float16)
            rhs[r + 3, c:c + 160] = k.astype(np.float16)
            rhs[r + 4, c:c + 160] = k2hi
            rhs[r + 5, c:c + 160] = k2lo
            rhs[r + 6, c:c + 160] = k2hi
        # pad cols: drive quad to +big via slot0's alpha_hi row
        rhs[4, base + 160 * nslots: base + NCOL] = np.float16(PAD_BIG)
    return rhs


def _lhs_table(centers, A, core):
    """Per-row coefficient matrix [KT, TILES*NG*128] fp16 for one core."""
    rows = np.arange(core * RPC, (core + 1) * RPC, dtype=np.int64)
    i = (rows // VOL).astype(np.float64)
    j = (rows % VOL).astype(np.float64)
    d0 = i[None, :] - centers[:, 0:1]          # [S, RPC]
    d1 = j[None, :] - centers[:, 1:2]
    c2 = centers[:, 2:3]
    a00 = A[:, 0, 0:1]; a01 = A[:, 0, 1:2]; a02 = A[:, 0, 2:3]
    a11 = A[:, 1, 1:2]; a12 = A[:, 1, 2:3]; a22 = A[:, 2, 2:3]
    lin = a02 * d0 + a12 * d1                  # [S, RPC]
    Q = 2.0 * lin - 2.0 * a22 * c2             # [S, RPC]
    C0 = a00 * d0 * d0 + 2.0 * a01 * d0 * d1 + a11 * d1 * d1 \
        - 2.0 * lin * c2 + a22 * c2 * c2       # [S, RPC]
    alpha = a22[:, 0]                          # [S]
    C0hi, C0lo = _split16(C0)
    Qhi, Qlo = _split16(Q)
    Ahi, Alo = _split16(alpha)

    lhs = np.zeros((KT, TILES * NG * 128), np.float16)
    for t in range(TILES):
        rsl = slice(t * 128, (t + 1) * 128)
        for g in range(NG):
            base = (t * NG + g) * 128
            nslots = 3 if g < 10 else 2
            for sl in range(nslots):
                s = 3 * g + sl
                r = 7 * sl
                lhs[r + 0, base:base + 128] = C0hi[s, rsl]
                lhs[r + 1, base:base + 128] = C0lo[s, rsl]
                lhs[r + 2, base:base + 128] = Qhi[s, rsl]
                lhs[r + 3, base:base + 128] = Qlo[s, rsl]
                lhs[r + 4, base:base + 128] = Ahi[s]
                lhs[r + 5, base:base + 128] = Ahi[s]
                lhs[r + 6, base:base + 128] = Alo[s]
    return lhs


def _build_nc():
    nc = bacc.Bacc("TRN2", target_bir_lowering=False, debug=False,
                   num_devices=N_CORES)
    rv = nc.declare_dram_parameter("rv", [RPC, VOL], F32, isOutput=False)
    om = nc.declare_dram_parameter("om", [RPC, VOL], F32, isOutput=False)
    lhs = nc.declare_dram_parameter("lhs", [KT, TILES * NG * 128], F16,
                                    isOutput=False)
    rhs = nc.declare_dram_parameter("rhs", [KT, NG * NCOL], F16, isOutput=False)
    partials = nc.declare_dram_parameter("partials", [1, 4], F32, isOutput=True)

    add = mybir.AluOpType.add
    mult = mybir.AluOpType.mult
    Exp = mybir.ActivationFunctionType.Exp
    Sigmoid = mybir.ActivationFunctionType.Sigmoid

    with ExitStack() as ctx:
        tc = ctx.enter_context(tile.TileContext(nc))
        cpool = ctx.enter_context(tc.tile_pool(name="const", bufs=1))
        ppool = ctx.enter_context(tc.tile_pool(name="psum", bufs=2, space="PSUM"))
        gpool = ctx.enter_context(tc.tile_pool(name="g", bufs=3))
        tpool = ctx.enter_context(tc.tile_pool(name="tmp", bufs=2))

        lhs_sb = cpool.tile([KT, TILES * NG * 128], F16)
        rhs_sb = cpool.tile([KT, NG * NCOL], F16)
        rv_sb = cpool.tile([128, TILES * VOL], F32)
        om_sb = cpool.tile([128, TILES * VOL], F32)
        rad = cpool.tile([128, TILES * VOL], F32)

        nc.sync.dma_start(lhs_sb[:], lhs[:])
        nc.sync.dma_start(rhs_sb[:], rhs[:])
        nc.sync.dma_start(
            rv_sb[:].rearrange("p (t k) -> p t k", t=TILES),
            rv[:].rearrange("(t p) k -> p t k", p=128))
        nc.sync.dma_start(
            om_sb[:].rearrange("p (t k) -> p t k", t=TILES),
            om[:].rearrange("(t p) k -> p t k", p=128))

        # Phase 1: rad[p, t*160+k] = sum_s exp(-quad_s)
        for t in range(TILES):
            rad_t = rad[:, t * VOL:(t + 1) * VOL]
            for ri, groups in enumerate(ROUND_GROUPS):
                n = len(groups)
                q = ppool.tile([128, 4 * NCOL], F32, tag="q")
                for gl, g in enumerate(groups):
                    nc.tensor.matmul(
                        q[:, gl * NCOL:(gl + 1) * NCOL],
                        lhsT=lhs_sb[:, (t * NG + g) * 128:(t * NG + g + 1) * 128],
                        rhs=rhs_sb[:, g * NCOL:(g + 1) * NCOL],
                        start=True, stop=True)
                gt = gpool.tile([128, 4 * NCOL], F32, tag="g")
                nc.scalar.activation(gt[:, :n * NCOL], q[:, :n * NCOL],
                                     Exp, scale=-1.0)
                red_src = (gt[:, :n * NCOL]
                           .rearrange("p (g c) -> p g c", g=n)[:, :, 0:480]
                           .rearrange("p g (s k) -> p k g s", s=3))
                if ri == 0:
                    nc.vector.tensor_reduce(rad_t, red_src,
                                            axis=mybir.AxisListType.XY, op=add)
                else:
                    tmp = tpool.tile([128, VOL], F32, tag="tmp")
                    nc.vector.tensor_reduce(tmp[:], red_src,
                                            axis=mybir.AxisListType.XY, op=add)
                    nc.vector.tensor_add(rad_t, rad_t, tmp[:])

        # Phase 2: masked sums.  E = sum(rad*rv), T = sum(rad),
        # EU = sum(sigmoid(rad*rv - 1)), U = sum(sigmoid(100*(rad*om - 0.5)))
        CH = 4
        CW = TILES * VOL // CH
        eparts = cpool.tile([128, CH], F32)
        tparts = cpool.tile([128, CH], F32)
        euparts = cpool.tile([128, CH], F32)
        uparts = cpool.tile([128, CH], F32)
        acc4 = cpool.tile([128, 4], F32)
        bneg1 = cpool.tile([128, 1], F32)
        bneg50 = cpool.tile([128, 1], F32)
        nc.vector.memset(bneg1[:], -1.0)
        nc.vector.memset(bneg50[:], -50.0)
        for c in range(CH):
            sl = slice(c * CW, (c + 1) * CW)
            eff = tpool.tile([128, CW], F32, tag="eff")
            nc.vector.scalar_tensor_tensor(
                eff[:], rad[:, sl], 1.0, rv_sb[:, sl], mult, mult,
                accum_out=eparts[:, c:c + 1])
            nc.scalar.activation(eff[:], eff[:], Sigmoid, bias=bneg1[:],
                                 scale=1.0, accum_out=euparts[:, c:c + 1])
            outv = tpool.tile([128, CW], F32, tag="outv")
            nc.vector.tensor_mul(outv[:], rad[:, sl], om_sb[:, sl])
            nc.scalar.activation(outv[:], outv[:], Sigmoid, bias=bneg50[:],
                                 scale=100.0, accum_out=uparts[:, c:c + 1])
            nc.vector.tensor_reduce(tparts[:, c:c + 1], rad[:, sl],
                                    axis=mybir.AxisListType.X, op=add)
        for idx, p4 in enumerate([eparts, tparts, euparts, uparts]):
            nc.vector.tensor_reduce(acc4[:, idx:idx + 1], p4[:],
                                    axis=mybir.AxisListType.X, op=add)
        ones = cpool.tile([128, 1], F32)
        nc.vector.memset(ones[:], 1.0)
        accq = ppool.tile([1, 4], F32, tag="q")
        nc.tensor.matmul(accq[:], lhsT=ones[:], rhs=acc4[:], start=True,
                         stop=True)
        res = tpool.tile([1, 4], F32, tag="res")
        nc.scalar.copy(res[:], accq[:])
        nc.sync.dma_start(partials[:], res[:])
    nc.compile()
    return nc


_NC_CACHE = {}
LAST_RESULT = None  # BassKernelResults of the most recent kernel() call


def _get_nc():
    if "nc" not in _NC_CACHE:
        _NC_CACHE["nc"] = _build_nc()
    return _NC_CACHE["nc"]


def kernel(x, radiation_volume, outside_mask):
    from concourse.bass_utils import run_bass_kernel_spmd

    nc = _get_nc()
    centers, A = _seed_params(x)
    rv2 = np.ascontiguousarray(
        np.asarray(radiation_volume, np.float32).reshape(ROWS, VOL))
    om2 = np.ascontiguousarray(
        np.asarray(outside_mask, np.float32).reshape(ROWS, VOL))
    rhs = _rhs_table()
    in_maps = []
    for c in range(N_CORES):
        in_maps.append({
            "rv": rv2[c * RPC:(c + 1) * RPC],
            "om": om2[c * RPC:(c + 1) * RPC],
            "lhs": np.ascontiguousarray(_lhs_table(centers, A, c)),
            "rhs": rhs,
        })
    out = run_bass_kernel_spmd(nc, in_maps, list(range(N_CORES)))
    global LAST_RESULT
    LAST_RESULT = out
    parts = np.stack([out.results[i]["partials"][0] for i in range(N_CORES)])
    E, T, EU, U = parts.sum(axis=0, dtype=np.float64)
    num_target = float(np.sum(rv2, dtype=np.float64))
    loss = (DVH_RATE - EU / num_target) + (1.0 - E / T) + U / num_target
    return np.array(loss, dtype=np.float32)


# revision 9
# speedup vs baseline: 1.0307x; 1.0307x over previous
# Culled Trainium2 Bass kernel for nn_DoseOptimizationLoss (v3).
#
# Same numeric core as v1 (fp16 hi/lo-split K=7/seed matmuls -> wide Exp ->
# strided-AP seed reduce -> masked-sigmoid epilogue), plus input-dependent
# culling:
#   * voxel rows regrouped into 16x8 spatial (i,j) blocks (128 rows = 1 tile)
#     so each seed's Gaussian support touches few tiles;
#   * per (block, seed): survives iff min_k quad <= CUT; non-survivors are
#     skipped entirely (exp(-quad) < e^-CUT contributes nothing);
#   * blocks LPT-assigned to cores to balance surviving work; each core's
#     blocks sorted by item count so the per-slot max across cores (the SPMD
#     program must be identical on all cores) wastes little;
#   * group slots padded with dummy seeds whose C0hi row is huge -> exp -> 0.
# The Bass program depends on x only through the 25 per-slot group counts,
# which are cached; tables/shards are per-input data.

import numpy as np

import concourse.bass as bass
import concourse.bacc as bacc
import concourse.mybir as mybir
import concourse.tile as tile
from contextlib import ExitStack

VOL = 160
S = 32
SIGMA = np.array([8.0, 4.0, 4.0])
N_CORES = 8
BI, BJ = 16, 8              # spatial block = 128 rows = one tile
NBLK = (VOL // BI) * (VOL // BJ)       # 200 blocks
TILES = NBLK // N_CORES                # 25 per core
KT = 21                     # matmul contraction rows (7 per seed slot)
NCOL = 512                  # moving cols per matmul = one PSUM bank
PAD_BIG = 60000.0
CUT = 22.0                  # drop (block, seed) with min quad > CUT
DVH_RATE = 0.9
F32 = mybir.dt.float32
F16 = mybir.dt.float16


def _seed_params(x):
    """float64 port of the reference's seed math: centers [S,3], A [S,3,3]."""
    xs = np.asarray(x, dtype=np.float64).reshape(S, 6)
    centers = xs[:, :3] * VOL
    d = xs[:, 3:]
    dot = d[:, 0]
    dot_c = np.clip(dot, -0.999999, 0.999999)
    angle = np.arccos(dot_c)
    z = np.zeros(S)
    axis_raw = np.stack([z, -d[:, 2], d[:, 1]], -1)
    nrm = np.linalg.norm(axis_raw, axis=-1, keepdims=True)
    axis = axis_raw / np.where(nrm > 1e-8, nrm, 1.0)
    cos_t = np.cos(angle)[:, None, None]
    sin_t = np.sin(angle)[:, None, None]
    a0, a1, a2 = axis[:, 0], axis[:, 1], axis[:, 2]
    K = np.stack([np.stack([z, -a2, a1], -1),
                  np.stack([a2, z, -a0], -1),
                  np.stack([-a1, a0, z], -1)], 1)
    eye = np.eye(3)
    R = cos_t * eye + (1.0 - cos_t) * axis[:, :, None] * axis[:, None, :] + sin_t * K
    R = np.where((np.abs(dot) >= 0.99)[:, None, None], eye, R)
    D = np.diag(1.0 / (2.0 * SIGMA ** 2))
    A = np.einsum('ski,kl,slj->sij', R, D, R)
    return centers, A


def _split16(v):
    hi = np.asarray(v, np.float64).astype(np.float16)
    lo = (np.asarray(v, np.float64) - hi.astype(np.float64)).astype(np.float16)
    return hi, lo


def _block_rows():
    """[NBLK, 128] global row ids (row = i*VOL + j) for each spatial block."""
    i = np.arange(VOL)
    j = np.arange(VOL)
    I, J = np.meshgrid(i, j, indexing='ij')
    rows = (I * VOL + J).reshape(VOL // BI, BI, VOL // BJ, BJ)
    return rows.transpose(0, 2, 1, 3).reshape(NBLK, 128)


def _row_coeffs(centers, A):
    """C0, Q [S, VOL*VOL] and alpha [S] for all rows, plus per-row min quad."""
    rows = np.arange(VOL * VOL)
    i = (rows // VOL).astype(np.float64)
    j = (rows % VOL).astype(np.float64)
    d0 = i[None, :] - centers[:, 0:1]
    d1 = j[None, :] - centers[:, 1:2]
    c2 = centers[:, 2:3]
    a00 = A[:, 0, 0:1]; a01 = A[:, 0, 1:2]; a02 = A[:, 0, 2:3]
    a11 = A[:, 1, 1:2]; a12 = A[:, 1, 2:3]; a22 = A[:, 2, 2:3]
    lin = a02 * d0 + a12 * d1
    Q = 2.0 * lin - 2.0 * a22 * c2
    C0 = a00 * d0 * d0 + 2.0 * a01 * d0 * d1 + a11 * d1 * d1 \
        - 2.0 * lin * c2 + a22 * c2 * c2
    alpha = a22[:, 0]
    mu = -Q / (2.0 * alpha[:, None])
    beta = C0 - Q * Q / (4.0 * alpha[:, None])
    qmin = np.where(mu < 0.0, C0,
                    np.where(mu > VOL - 1.0,
                             C0 + Q * (VOL - 1.0) + alpha[:, None] * (VOL - 1.0) ** 2,
                             beta))
    return C0, Q, alpha, qmin


def plan(x):
    """Input-dependent schedule.

    Returns dict with:
      blocks_of_core [N_CORES, TILES] block ids (slot order),
      seeds_of      {(core, slot): [seed ids]},
      G             [TILES] group count per slot (same for all cores),
    """
    centers, A = _seed_params(x)
    C0, Q, alpha, qmin = _row_coeffs(centers, A)
    brows = _block_rows()                       # [NBLK, 128]
    # surviving seeds per block
    qmin_blk = qmin[:, brows].min(axis=2)       # [S, NBLK]
    alive = qmin_blk <= CUT                     # [S, NBLK]
    counts = alive.sum(axis=0)                  # [NBLK]

    # LPT assignment: 25 blocks per core, balancing total item count
    order = np.argsort(-counts, kind='stable')
    loads = np.zeros(N_CORES, dtype=int)
    nblk = np.zeros(N_CORES, dtype=int)
    blocks_of_core = [[] for _ in range(N_CORES)]
    for b in order:
        elig = [c for c in range(N_CORES) if nblk[c] < TILES]
        c = min(elig, key=lambda c: (loads[c], c))
        blocks_of_core[c].append(int(b))
        loads[c] += int(counts[b])
        nblk[c] += 1
    # slot order: per core, descending item count
    for c in range(N_CORES):
        blocks_of_core[c].sort(key=lambda b: -int(counts[b]))
    # per-slot group counts (max over cores)
    G = []
    for t in range(TILES):
        m = max(int(counts[blocks_of_core[c][t]]) for c in range(N_CORES))
        G.append((m + 2) // 3)
    seeds_of = {}
    for c in range(N_CORES):
        for t in range(TILES):
            b = blocks_of_core[c][t]
            seeds_of[(c, t)] = [int(s) for s in np.nonzero(alive[:, b])[0]]
    return {
        "blocks_of_core": np.array([bc for bc in blocks_of_core]),
        "seeds_of": seeds_of,
        "G": G,
        "centers": centers,
        "A": A,
    }


def _rhs_table():
    """Single k-pattern block [KT, NCOL] shared by every group.

    Columns are (k, s)-interleaved: col = 3*k + s for k in [0,160), seed
    slot s in [0,3) — so the seed fold in the reduce reads contiguously.
    """
    k = np.arange(VOL, dtype=np.float64)
    k2hi, k2lo = _split16(k * k)
    rhs = np.zeros((KT, NCOL), np.float16)
    for sl in range(3):
        r = 7 * sl
        cols = slice(sl, 480, 3)
        rhs[r + 0, cols] = np.float16(1.0)
        rhs[r + 1, cols] = np.float16(1.0)
        rhs[r + 2, cols] = k.astype(np.float16)
        rhs[r + 3, cols] = k.astype(np.float16)
        rhs[r + 4, cols] = k2hi
        rhs[r + 5, cols] = k2lo
        rhs[r + 6, cols] = k2hi
    rhs[0, 480:NCOL] = np.float16(PAD_BIG)  # pad cols via slot0 C0hi row
    return rhs


def _lhs_table(pl, core):
    """[KT, GT*128] fp16 coefficient matrix for one core (GT = sum(G))."""
    centers, A = pl["centers"], pl["A"]
    G = pl["G"]
    GT = sum(G)
    brows = _block_rows()
    C0, Q, alpha, _ = _row_coeffs(centers, A)
    C0hi, C0lo = _split16(C0)
    Qhi, Qlo = _split16(Q)
    Ahi, Alo = _split16(alpha)

    lhs = np.zeros((KT, max(GT, 1) * 128), np.float16)
    if GT == 0:
        lhs[0, :] = np.float16(PAD_BIG)
    gidx = 0
    for t in range(TILES):
        b = pl["blocks_of_core"][core][t]
        rows = brows[b]                        # [128] global row ids
        seeds = pl["seeds_of"][(core, t)]
        for g in range(G[t]):
            base = gidx * 128
            for sl in range(3):
                si = g * 3 + sl
                r = 7 * sl
                if si < len(seeds):
                    s = seeds[si]
                    lhs[r + 0, base:base + 128] = C0hi[s, rows]
                    lhs[r + 1, base:base + 128] = C0lo[s, rows]
                    lhs[r + 2, base:base + 128] = Qhi[s, rows]
                    lhs[r + 3, base:base + 128] = Qlo[s, rows]
                    lhs[r + 4, base:base + 128] = Ahi[s]
                    lhs[r + 5, base:base + 128] = Ahi[s]
                    lhs[r + 6, base:base + 128] = Alo[s]
                else:
                    lhs[r + 0, base:base + 128] = np.float16(PAD_BIG)
            gidx += 1
    return lhs


def _build_nc(G):
    """Build the SPMD program for per-slot group counts G (len TILES)."""
    GT = sum(G)
    nc = bacc.Bacc("TRN2", target_bir_lowering=False, debug=False,
                   num_devices=N_CORES)
    rv = nc.declare_dram_parameter("rv", [TILES * 128, VOL], F32, isOutput=False)
    om = nc.declare_dram_parameter("om", [TILES * 128, VOL], F32, isOutput=False)
    lhs = nc.declare_dram_parameter("lhs", [KT, max(GT, 1) * 128], F16,
                                    isOutput=False)
    rhs = nc.declare_dram_parameter("rhs", [KT, NCOL], F16, isOutput=False)
    partials = nc.declare_dram_parameter("partials", [1, 4], F32, isOutput=True)

    add = mybir.AluOpType.add
    mult = mybir.AluOpType.mult
    Exp = mybir.ActivationFunctionType.Exp
    Sigmoid = mybir.ActivationFunctionType.Sigmoid

    # (tile, local group) stream in program order, packed into PSUM fills of 4
    work = [(t, g) for t in range(TILES) for g in range(G[t])]

    with ExitStack() as ctx:
        tc = ctx.enter_context(tile.TileContext(nc))
        cpool = ctx.enter_context(tc.tile_pool(name="const", bufs=1))
        ppool = ctx.enter_context(tc.tile_pool(name="psum", bufs=2, space="PSUM"))
        gpool = ctx.enter_context(tc.tile_pool(name="g", bufs=3))
        tpool = ctx.enter_context(tc.tile_pool(name="tmp", bufs=4))

        lhs_sb = cpool.tile([KT, max(GT, 1) * 128], F16)
        rhs_sb = cpool.tile([KT, NCOL], F16)
        rv_sb = cpool.tile([128, TILES * VOL], F32)
        om_sb = cpool.tile([128, TILES * VOL], F32)
        rad = cpool.tile([128, TILES * VOL], F32)

        nc.sync.dma_start(rhs_sb[:], rhs[:])
        # split the lhs table DMA so early fills aren't gated on the tail
        nsplit = 4
        cols = max(GT, 1) * 128
        step = ((cols + nsplit - 1) // nsplit + 127) // 128 * 128
        for o in range(0, cols, step):
            e = min(o + step, cols)
            nc.sync.dma_start(lhs_sb[:, o:e], lhs[:, o:e])
        nc.sync.dma_start(
            rv_sb[:].rearrange("p (t k) -> p t k", t=TILES),
            rv[:].rearrange("(t p) k -> p t k", p=128))
        nc.sync.dma_start(
            om_sb[:].rearrange("p (t k) -> p t k", t=TILES),
            om[:].rearrange("(t p) k -> p t k", p=128))

        # tiles with no groups: zero their rad slice
        written = set()
        for t in range(TILES):
            if G[t] == 0:
                nc.vector.memset(rad[:, t * VOL:(t + 1) * VOL], 0.0)
                written.add(t)

        # Phase 1: stream groups through PSUM fills of up to 4.  The
        # seed-sum alternates between VectorE (fused 4D strided reduce)
        # and GpSimd (block adds) to split the DVE bottleneck.
        nfills = (len(work) + 3) // 4
        for fi, f0 in enumerate(range(0, len(work), 4)):
            fill = work[f0:f0 + 4]
            n = len(fill)
            q = ppool.tile([128, 4 * NCOL], F32, tag="q")
            for gl, (t, g) in enumerate(fill):
                gi = f0 + gl   # global group index = lhs slot
                nc.tensor.matmul(
                    q[:, gl * NCOL:(gl + 1) * NCOL],
                    lhsT=lhs_sb[:, gi * 128:(gi + 1) * 128],
                    rhs=rhs_sb[:],
                    start=True, stop=True)
            # Exp skips the 32 pad cols of each 512 block: strided PSUM
            # read, packed 480-wide SBUF write.
            gt = gpool.tile([128, 4 * 480], F32, tag="g")
            q_src = (q[:, :n * NCOL]
                     .rearrange("p (g c) -> p g c", g=n)[:, :, 0:480])
            nc.scalar.activation(
                gt[:, :n * 480].rearrange("p (g c) -> p g c", g=n),
                q_src, Exp, scale=-1.0)
            use_gpsimd = (fi % 5 in (1, 3))
            # accumulate per contiguous same-tile run within the fill
            i0 = 0
            while i0 < n:
                t = fill[i0][0]
                i1 = i0
                while i1 < n and fill[i1][0] == t:
                    i1 += 1
                m = i1 - i0
                rad_t = rad[:, t * VOL:(t + 1) * VOL]
                if not use_gpsimd:
                    # cols of each 480-chunk are (k, s) interleaved
                    red_src = (gt[:, i0 * 480:i1 * 480]
                               .rearrange("p (g k s) -> p k g s", g=m, s=3))
                    if t not in written:
                        nc.vector.tensor_reduce(
                            rad_t, red_src, axis=mybir.AxisListType.XY, op=add)
                        written.add(t)
                    else:
                        tmp = tpool.tile([128, VOL], F32, tag="tmp")
                        nc.vector.tensor_reduce(
                            tmp[:], red_src, axis=mybir.AxisListType.XY, op=add)
                        nc.vector.tensor_add(rad_t, rad_t, tmp[:])
                else:
                    # GpSimd: wide adds over the 480-chunks, then a
                    # strided (k,s) fold
                    eng = nc.gpsimd
                    src = gt[:, i0 * 480:i1 * 480]
                    if m == 1:
                        ssum = src[:, 0:480]
                    elif m == 2:
                        bt = tpool.tile([128, 480], F32, tag="gb")
                        eng.tensor_add(bt[:], src[:, 0:480], src[:, 480:960])
                        ssum = bt[:]
                    elif m == 3:
                        bt = tpool.tile([128, 480], F32, tag="gb")
                        eng.tensor_add(bt[:], src[:, 0:480], src[:, 960:1440])
                        bt2 = tpool.tile([128, 480], F32, tag="gb2")
                        eng.tensor_add(bt2[:], bt[:], src[:, 480:960])
                        ssum = bt2[:]
                    else:  # m == 4
                        bt = tpool.tile([128, 960], F32, tag="gbw")
                        eng.tensor_add(bt[:], src[:, 0:960], src[:, 960:1920])
                        bt2 = tpool.tile([128, 480], F32, tag="gb2")
                        eng.tensor_add(bt2[:], bt[:, 0:480], bt[:, 480:960])
                        ssum = bt2[:]
                    ks = ssum.rearrange("p (k s) -> p k s", s=3)
                    u = tpool.tile([128, VOL], F32, tag="gu")
                    eng.tensor_add(u[:], ks[:, :, 0], ks[:, :, 1])
                    if t not in written:
                        eng.tensor_add(rad_t, u[:], ks[:, :, 2])
                        written.add(t)
                    else:
                        u2 = tpool.tile([128, VOL], F32, tag="gu2")
                        eng.tensor_add(u2[:], u[:], ks[:, :, 2])
                        eng.tensor_add(rad_t, rad_t, u2[:])
                i0 = i1

        # Phase 2: masked sums
        CH = 4
        CW = TILES * VOL // CH
        eparts = cpool.tile([128, CH], F32)
        tparts = cpool.tile([128, CH], F32)
        euparts = cpool.tile([128, CH], F32)
        uparts = cpool.tile([128, CH], F32)
        acc4 = cpool.tile([128, 4], F32)
        bneg1 = cpool.tile([128, 1], F32)
        bneg50 = cpool.tile([128, 1], F32)
        nc.vector.memset(bneg1[:], -1.0)
        nc.vector.memset(bneg50[:], -50.0)
        for c in range(CH):
            sl = slice(c * CW, (c + 1) * CW)
            eff = tpool.tile([128, CW], F32, tag="eff")
            nc.vector.scalar_tensor_tensor(
                eff[:], rad[:, sl], 1.0, rv_sb[:, sl], mult, mult,
                accum_out=eparts[:, c:c + 1])
            nc.scalar.activation(eff[:], eff[:], Sigmoid, bias=bneg1[:],
                                 scale=1.0, accum_out=euparts[:, c:c + 1])
            outv = tpool.tile([128, CW], F32, tag="outv")
            nc.vector.tensor_mul(outv[:], rad[:, sl], om_sb[:, sl])
            nc.scalar.activation(outv[:], outv[:], Sigmoid, bias=bneg50[:],
                                 scale=100.0, accum_out=uparts[:, c:c + 1])
            nc.vector.tensor_reduce(tparts[:, c:c + 1], rad[:, sl],
                                    axis=mybir.AxisListType.X, op=add)
        for idx, p4 in enumerate([eparts, tparts, euparts, uparts]):
            nc.vector.tensor_reduce(acc4[:, idx:idx + 1], p4[:],
                                    axis=mybir.AxisListType.X, op=add)
        ones = cpool.tile([128, 1], F32)
        nc.vector.memset(ones[:], 1.0)
        accq = ppool.tile([1, 4], F32, tag="q")
        nc.tensor.matmul(accq[:], lhsT=ones[:], rhs=acc4[:], start=True,
                         stop=True)
        res = tpool.tile([1, 4], F32, tag="res")
        nc.scalar.copy(res[:], accq[:])
        nc.sync.dma_start(partials[:], res[:])
    nc.compile()
    return nc


_NC_CACHE = {}
LAST_RESULT = None


def _get_nc(G):
    key = tuple(G)
    if key not in _NC_CACHE:
        _NC_CACHE[key] = _build_nc(list(G))
    return _NC_CACHE[key]


def kernel(x, radiation_volume, outside_mask):
    from concourse.bass_utils import run_bass_kernel_spmd

    pl = plan(x)
    nc = _get_nc(pl["G"])
    rv2 = np.asarray(radiation_volume, np.float32).reshape(VOL * VOL, VOL)
    om2 = np.asarray(outside_mask, np.float32).reshape(VOL * VOL, VOL)
    brows = _block_rows()
    rhs = _rhs_table()
    in_maps = []
    for c in range(N_CORES):
        rows = brows[pl["blocks_of_core"][c]].reshape(-1)   # [3200]
        in_maps.append({
            "rv": np.ascontiguousarray(rv2[rows]),
            "om": np.ascontiguousarray(om2[rows]),
            "lhs": np.ascontiguousarray(_lhs_table(pl, c)),
            "rhs": rhs,
        })
    out = run_bass_kernel_spmd(nc, in_maps, list(range(N_CORES)))
    global LAST_RESULT
    LAST_RESULT = out
    parts = np.stack([out.results[i]["partials"][0] for i in range(N_CORES)])
    E, T, EU, U = parts.sum(axis=0, dtype=np.float64)
    num_target = float(np.sum(rv2, dtype=np.float64))
    loss = (DVH_RATE - EU / num_target) + (1.0 - E / T) + U / num_target
    return np.array(loss, dtype=np.float32)


# revision 10
# speedup vs baseline: 1.0997x; 1.0670x over previous
# Culled Trainium2 Bass kernel for nn_DoseOptimizationLoss (v3).
#
# Same numeric core as v1 (fp16 hi/lo-split K=7/seed matmuls -> wide Exp ->
# strided-AP seed reduce -> masked-sigmoid epilogue), plus input-dependent
# culling:
#   * voxel rows regrouped into 16x8 spatial (i,j) blocks (128 rows = 1 tile)
#     so each seed's Gaussian support touches few tiles;
#   * per (block, seed): survives iff min_k quad <= CUT; non-survivors are
#     skipped entirely (exp(-quad) < e^-CUT contributes nothing);
#   * blocks LPT-assigned to cores to balance surviving work; each core's
#     blocks sorted by item count so the per-slot max across cores (the SPMD
#     program must be identical on all cores) wastes little;
#   * group slots padded with dummy seeds whose C0hi row is huge -> exp -> 0.
# The Bass program depends on x only through the 25 per-slot group counts,
# which are cached; tables/shards are per-input data.

import numpy as np

import concourse.bass as bass
import concourse.bacc as bacc
import concourse.mybir as mybir
import concourse.tile as tile
from contextlib import ExitStack

VOL = 160
S = 32
SIGMA = np.array([8.0, 4.0, 4.0])
N_CORES = 8
BI, BJ = 16, 8              # spatial block = 128 rows = one tile
NBLK = (VOL // BI) * (VOL // BJ)       # 200 blocks
TILES = NBLK // N_CORES                # 25 per core
KT = 21                     # matmul contraction rows (7 per seed slot)
NCOL = 512                  # moving cols per matmul = one PSUM bank
PAD_BIG = 60000.0
CUT = 16.0                  # drop (block, seed) with min quad > CUT
DVH_RATE = 0.9
F32 = mybir.dt.float32
F16 = mybir.dt.float16


def _seed_params(x):
    """float64 port of the reference's seed math: centers [S,3], A [S,3,3]."""
    xs = np.asarray(x, dtype=np.float64).reshape(S, 6)
    centers = xs[:, :3] * VOL
    d = xs[:, 3:]
    dot = d[:, 0]
    dot_c = np.clip(dot, -0.999999, 0.999999)
    angle = np.arccos(dot_c)
    z = np.zeros(S)
    axis_raw = np.stack([z, -d[:, 2], d[:, 1]], -1)
    nrm = np.linalg.norm(axis_raw, axis=-1, keepdims=True)
    axis = axis_raw / np.where(nrm > 1e-8, nrm, 1.0)
    cos_t = np.cos(angle)[:, None, None]
    sin_t = np.sin(angle)[:, None, None]
    a0, a1, a2 = axis[:, 0], axis[:, 1], axis[:, 2]
    K = np.stack([np.stack([z, -a2, a1], -1),
                  np.stack([a2, z, -a0], -1),
                  np.stack([-a1, a0, z], -1)], 1)
    eye = np.eye(3)
    R = cos_t * eye + (1.0 - cos_t) * axis[:, :, None] * axis[:, None, :] + sin_t * K
    R = np.where((np.abs(dot) >= 0.99)[:, None, None], eye, R)
    D = np.diag(1.0 / (2.0 * SIGMA ** 2))
    A = np.einsum('ski,kl,slj->sij', R, D, R)
    return centers, A


def _split16(v):
    hi = np.asarray(v, np.float64).astype(np.float16)
    lo = (np.asarray(v, np.float64) - hi.astype(np.float64)).astype(np.float16)
    return hi, lo


def _block_rows():
    """[NBLK, 128] global row ids (row = i*VOL + j) for each spatial block."""
    i = np.arange(VOL)
    j = np.arange(VOL)
    I, J = np.meshgrid(i, j, indexing='ij')
    rows = (I * VOL + J).reshape(VOL // BI, BI, VOL // BJ, BJ)
    return rows.transpose(0, 2, 1, 3).reshape(NBLK, 128)


def _row_coeffs(centers, A):
    """C0, Q [S, VOL*VOL] and alpha [S] for all rows, plus per-row min quad."""
    rows = np.arange(VOL * VOL)
    i = (rows // VOL).astype(np.float64)
    j = (rows % VOL).astype(np.float64)
    d0 = i[None, :] - centers[:, 0:1]
    d1 = j[None, :] - centers[:, 1:2]
    c2 = centers[:, 2:3]
    a00 = A[:, 0, 0:1]; a01 = A[:, 0, 1:2]; a02 = A[:, 0, 2:3]
    a11 = A[:, 1, 1:2]; a12 = A[:, 1, 2:3]; a22 = A[:, 2, 2:3]
    lin = a02 * d0 + a12 * d1
    Q = 2.0 * lin - 2.0 * a22 * c2
    C0 = a00 * d0 * d0 + 2.0 * a01 * d0 * d1 + a11 * d1 * d1 \
        - 2.0 * lin * c2 + a22 * c2 * c2
    alpha = a22[:, 0]
    mu = -Q / (2.0 * alpha[:, None])
    beta = C0 - Q * Q / (4.0 * alpha[:, None])
    qmin = np.where(mu < 0.0, C0,
                    np.where(mu > VOL - 1.0,
                             C0 + Q * (VOL - 1.0) + alpha[:, None] * (VOL - 1.0) ** 2,
                             beta))
    return C0, Q, alpha, qmin


def plan(x):
    """Input-dependent schedule.

    Returns dict with:
      blocks_of_core [N_CORES, TILES] block ids (slot order),
      seeds_of      {(core, slot): [seed ids]},
      G             [TILES] group count per slot (same for all cores),
    """
    centers, A = _seed_params(x)
    C0, Q, alpha, qmin = _row_coeffs(centers, A)
    brows = _block_rows()                       # [NBLK, 128]
    # surviving seeds per block
    qmin_blk = qmin[:, brows].min(axis=2)       # [S, NBLK]
    alive = qmin_blk <= CUT                     # [S, NBLK]
    counts = alive.sum(axis=0)                  # [NBLK]

    # LPT assignment: 25 blocks per core, balancing total item count
    order = np.argsort(-counts, kind='stable')
    loads = np.zeros(N_CORES, dtype=int)
    nblk = np.zeros(N_CORES, dtype=int)
    blocks_of_core = [[] for _ in range(N_CORES)]
    for b in order:
        elig = [c for c in range(N_CORES) if nblk[c] < TILES]
        c = min(elig, key=lambda c: (loads[c], c))
        blocks_of_core[c].append(int(b))
        loads[c] += int(counts[b])
        nblk[c] += 1
    # slot order: per core, descending item count
    for c in range(N_CORES):
        blocks_of_core[c].sort(key=lambda b: -int(counts[b]))
    # per-slot group counts (max over cores)
    G = []
    for t in range(TILES):
        m = max(int(counts[blocks_of_core[c][t]]) for c in range(N_CORES))
        G.append((m + 2) // 3)
    seeds_of = {}
    for c in range(N_CORES):
        for t in range(TILES):
            b = blocks_of_core[c][t]
            seeds_of[(c, t)] = [int(s) for s in np.nonzero(alive[:, b])[0]]
    return {
        "blocks_of_core": np.array([bc for bc in blocks_of_core]),
        "seeds_of": seeds_of,
        "G": G,
        "centers": centers,
        "A": A,
    }


def _rhs_table():
    """Single k-pattern block [KT, NCOL] shared by every group.

    Columns are (k, s)-interleaved: col = 3*k + s for k in [0,160), seed
    slot s in [0,3) — so the seed fold in the reduce reads contiguously.
    """
    k = np.arange(VOL, dtype=np.float64)
    k2hi, k2lo = _split16(k * k)
    rhs = np.zeros((KT, NCOL), np.float16)
    for sl in range(3):
        r = 7 * sl
        cols = slice(sl, 480, 3)
        rhs[r + 0, cols] = np.float16(1.0)
        rhs[r + 1, cols] = np.float16(1.0)
        rhs[r + 2, cols] = k.astype(np.float16)
        rhs[r + 3, cols] = k.astype(np.float16)
        rhs[r + 4, cols] = k2hi
        rhs[r + 5, cols] = k2lo
        rhs[r + 6, cols] = k2hi
    rhs[0, 480:NCOL] = np.float16(PAD_BIG)  # pad cols via slot0 C0hi row
    return rhs


def _lhs_table(pl, core):
    """[KT, GT*128] fp16 coefficient matrix for one core (GT = sum(G))."""
    centers, A = pl["centers"], pl["A"]
    G = pl["G"]
    GT = sum(G)
    brows = _block_rows()
    C0, Q, alpha, _ = _row_coeffs(centers, A)
    C0hi, C0lo = _split16(C0)
    Qhi, Qlo = _split16(Q)
    Ahi, Alo = _split16(alpha)

    lhs = np.zeros((KT, max(GT, 1) * 128), np.float16)
    if GT == 0:
        lhs[0, :] = np.float16(PAD_BIG)
    gidx = 0
    for t in range(TILES):
        b = pl["blocks_of_core"][core][t]
        rows = brows[b]                        # [128] global row ids
        seeds = pl["seeds_of"][(core, t)]
        for g in range(G[t]):
            base = gidx * 128
            for sl in range(3):
                si = g * 3 + sl
                r = 7 * sl
                if si < len(seeds):
                    s = seeds[si]
                    lhs[r + 0, base:base + 128] = C0hi[s, rows]
                    lhs[r + 1, base:base + 128] = C0lo[s, rows]
                    lhs[r + 2, base:base + 128] = Qhi[s, rows]
                    lhs[r + 3, base:base + 128] = Qlo[s, rows]
                    lhs[r + 4, base:base + 128] = Ahi[s]
                    lhs[r + 5, base:base + 128] = Ahi[s]
                    lhs[r + 6, base:base + 128] = Alo[s]
                else:
                    lhs[r + 0, base:base + 128] = np.float16(PAD_BIG)
            gidx += 1
    return lhs


def _build_nc(G):
    """Build the SPMD program for per-slot group counts G (len TILES)."""
    GT = sum(G)
    nc = bacc.Bacc("TRN2", target_bir_lowering=False, debug=False,
                   num_devices=N_CORES)
    rv = nc.declare_dram_parameter("rv", [TILES * 128, VOL], F32, isOutput=False)
    om = nc.declare_dram_parameter("om", [TILES * 128, VOL], F32, isOutput=False)
    lhs = nc.declare_dram_parameter("lhs", [KT, max(GT, 1) * 128], F16,
                                    isOutput=False)
    rhs = nc.declare_dram_parameter("rhs", [KT, NCOL], F16, isOutput=False)
    partials = nc.declare_dram_parameter("partials", [1, 4], F32, isOutput=True)

    add = mybir.AluOpType.add
    mult = mybir.AluOpType.mult
    Exp = mybir.ActivationFunctionType.Exp
    Sigmoid = mybir.ActivationFunctionType.Sigmoid

    # (tile, local group) stream in program order, packed into PSUM fills of 4
    work = [(t, g) for t in range(TILES) for g in range(G[t])]

    with ExitStack() as ctx:
        tc = ctx.enter_context(tile.TileContext(nc))
        cpool = ctx.enter_context(tc.tile_pool(name="const", bufs=1))
        ppool = ctx.enter_context(tc.tile_pool(name="psum", bufs=2, space="PSUM"))
        gpool = ctx.enter_context(tc.tile_pool(name="g", bufs=3))
        tpool = ctx.enter_context(tc.tile_pool(name="tmp", bufs=4))

        lhs_sb = cpool.tile([KT, max(GT, 1) * 128], F16)
        rhs_sb = cpool.tile([KT, NCOL], F16)
        rv_sb = cpool.tile([128, TILES * VOL], F32)
        om_sb = cpool.tile([128, TILES * VOL], F32)
        rad = cpool.tile([128, TILES * VOL], F32)

        nc.sync.dma_start(rhs_sb[:], rhs[:])
        # split the lhs table DMA so early fills aren't gated on the tail
        nsplit = 4
        cols = max(GT, 1) * 128
        step = ((cols + nsplit - 1) // nsplit + 127) // 128 * 128
        for o in range(0, cols, step):
            e = min(o + step, cols)
            nc.sync.dma_start(lhs_sb[:, o:e], lhs[:, o:e])
        nc.sync.dma_start(
            rv_sb[:].rearrange("p (t k) -> p t k", t=TILES),
            rv[:].rearrange("(t p) k -> p t k", p=128))
        nc.sync.dma_start(
            om_sb[:].rearrange("p (t k) -> p t k", t=TILES),
            om[:].rearrange("(t p) k -> p t k", p=128))

        # tiles with no groups: zero their rad slice
        written = set()
        for t in range(TILES):
            if G[t] == 0:
                nc.vector.memset(rad[:, t * VOL:(t + 1) * VOL], 0.0)
                written.add(t)

        # Phase 1: stream groups through PSUM fills of up to 4.  The
        # seed-sum alternates between VectorE (fused 4D strided reduce)
        # and GpSimd (block adds) to split the DVE bottleneck.
        nfills = (len(work) + 3) // 4
        for fi, f0 in enumerate(range(0, len(work), 4)):
            fill = work[f0:f0 + 4]
            n = len(fill)
            q = ppool.tile([128, 4 * NCOL], F32, tag="q")
            for gl, (t, g) in enumerate(fill):
                gi = f0 + gl   # global group index = lhs slot
                nc.tensor.matmul(
                    q[:, gl * NCOL:(gl + 1) * NCOL],
                    lhsT=lhs_sb[:, gi * 128:(gi + 1) * 128],
                    rhs=rhs_sb[:],
                    start=True, stop=True)
            # Exp skips the 32 pad cols of each 512 block: strided PSUM
            # read, packed 480-wide SBUF write.
            gt = gpool.tile([128, 4 * 480], F32, tag="g")
            q_src = (q[:, :n * NCOL]
                     .rearrange("p (g c) -> p g c", g=n)[:, :, 0:480])
            nc.scalar.activation(
                gt[:, :n * 480].rearrange("p (g c) -> p g c", g=n),
                q_src, Exp, scale=-1.0)
            use_gpsimd = (fi % 5 in (1, 3))
            # accumulate per contiguous same-tile run within the fill
            i0 = 0
            while i0 < n:
                t = fill[i0][0]
                i1 = i0
                while i1 < n and fill[i1][0] == t:
                    i1 += 1
                m = i1 - i0
                rad_t = rad[:, t * VOL:(t + 1) * VOL]
                if not use_gpsimd:
                    # cols of each 480-chunk are (k, s) interleaved
                    red_src = (gt[:, i0 * 480:i1 * 480]
                               .rearrange("p (g k s) -> p k g s", g=m, s=3))
                    if t not in written:
                        nc.vector.tensor_reduce(
                            rad_t, red_src, axis=mybir.AxisListType.XY, op=add)
                        written.add(t)
                    else:
                        tmp = tpool.tile([128, VOL], F32, tag="tmp")
                        nc.vector.tensor_reduce(
                            tmp[:], red_src, axis=mybir.AxisListType.XY, op=add)
                        nc.vector.tensor_add(rad_t, rad_t, tmp[:])
                else:
                    # GpSimd: wide adds over the 480-chunks, then a
                    # strided (k,s) fold
                    eng = nc.gpsimd
                    src = gt[:, i0 * 480:i1 * 480]
                    if m == 1:
                        ssum = src[:, 0:480]
                    elif m == 2:
                        bt = tpool.tile([128, 480], F32, tag="gb")
                        eng.tensor_add(bt[:], src[:, 0:480], src[:, 480:960])
                        ssum = bt[:]
                    elif m == 3:
                        bt = tpool.tile([128, 480], F32, tag="gb")
                        eng.tensor_add(bt[:], src[:, 0:480], src[:, 960:1440])
                        bt2 = tpool.tile([128, 480], F32, tag="gb2")
                        eng.tensor_add(bt2[:], bt[:], src[:, 480:960])
                        ssum = bt2[:]
                    else:  # m == 4
                        bt = tpool.tile([128, 960], F32, tag="gbw")
                        eng.tensor_add(bt[:], src[:, 0:960], src[:, 960:1920])
                        bt2 = tpool.tile([128, 480], F32, tag="gb2")
                        eng.tensor_add(bt2[:], bt[:, 0:480], bt[:, 480:960])
                        ssum = bt2[:]
                    ks = ssum.rearrange("p (k s) -> p k s", s=3)
                    u = tpool.tile([128, VOL], F32, tag="gu")
                    eng.tensor_add(u[:], ks[:, :, 0], ks[:, :, 1])
                    if t not in written:
                        eng.tensor_add(rad_t, u[:], ks[:, :, 2])
                        written.add(t)
                    else:
                        u2 = tpool.tile([128, VOL], F32, tag="gu2")
                        eng.tensor_add(u2[:], u[:], ks[:, :, 2])
                        eng.tensor_add(rad_t, rad_t, u2[:])
                i0 = i1

        # Phase 2: masked sums
        CH = 2
        CW = TILES * VOL // CH
        eparts = cpool.tile([128, CH], F32)
        tparts = cpool.tile([128, CH], F32)
        euparts = cpool.tile([128, CH], F32)
        uparts = cpool.tile([128, CH], F32)
        acc4 = cpool.tile([128, 4], F32)
        bneg1 = cpool.tile([128, 1], F32)
        bneg50 = cpool.tile([128, 1], F32)
        nc.vector.memset(bneg1[:], -1.0)
        nc.vector.memset(bneg50[:], -50.0)
        for c in range(CH):
            sl = slice(c * CW, (c + 1) * CW)
            eff = tpool.tile([128, CW], F32, tag="eff")
            nc.vector.scalar_tensor_tensor(
                eff[:], rad[:, sl], 1.0, rv_sb[:, sl], mult, mult,
                accum_out=eparts[:, c:c + 1])
            nc.scalar.activation(eff[:], eff[:], Sigmoid, bias=bneg1[:],
                                 scale=1.0, accum_out=euparts[:, c:c + 1])
            outv = tpool.tile([128, CW], F32, tag="outv")
            nc.vector.tensor_mul(outv[:], rad[:, sl], om_sb[:, sl])
            nc.scalar.activation(outv[:], outv[:], Sigmoid, bias=bneg50[:],
                                 scale=100.0, accum_out=uparts[:, c:c + 1])
            nc.vector.tensor_reduce(tparts[:, c:c + 1], rad[:, sl],
                                    axis=mybir.AxisListType.X, op=add)
        for idx, p4 in enumerate([eparts, tparts, euparts, uparts]):
            nc.vector.tensor_reduce(acc4[:, idx:idx + 1], p4[:],
                                    axis=mybir.AxisListType.X, op=add)
        ones = cpool.tile([128, 1], F32)
        nc.vector.memset(ones[:], 1.0)
        accq = ppool.tile([1, 4], F32, tag="q")
        nc.tensor.matmul(accq[:], lhsT=ones[:], rhs=acc4[:], start=True,
                         stop=True)
        res = tpool.tile([1, 4], F32, tag="res")
        nc.scalar.copy(res[:], accq[:])
        nc.sync.dma_start(partials[:], res[:])
    nc.compile()
    return nc


_NC_CACHE = {}
LAST_RESULT = None


def _get_nc(G):
    key = tuple(G)
    if key not in _NC_CACHE:
        _NC_CACHE[key] = _build_nc(list(G))
    return _NC_CACHE[key]


def kernel(x, radiation_volume, outside_mask):
    from concourse.bass_utils import run_bass_kernel_spmd

    pl = plan(x)
    nc = _get_nc(pl["G"])
    rv2 = np.asarray(radiation_volume, np.float32).reshape(VOL * VOL, VOL)
    om2 = np.asarray(outside_mask, np.float32).reshape(VOL * VOL, VOL)
    brows = _block_rows()
    rhs = _rhs_table()
    in_maps = []
    for c in range(N_CORES):
        rows = brows[pl["blocks_of_core"][c]].reshape(-1)   # [3200]
        in_maps.append({
            "rv": np.ascontiguousarray(rv2[rows]),
            "om": np.ascontiguousarray(om2[rows]),
            "lhs": np.ascontiguousarray(_lhs_table(pl, c)),
            "rhs": rhs,
        })
    out = run_bass_kernel_spmd(nc, in_maps, list(range(N_CORES)))
    global LAST_RESULT
    LAST_RESULT = out
    parts = np.stack([out.results[i]["partials"][0] for i in range(N_CORES)])
    E, T, EU, U = parts.sum(axis=0, dtype=np.float64)
    num_target = float(np.sum(rv2, dtype=np.float64))
    loss = (DVH_RATE - EU / num_target) + (1.0 - E / T) + U / num_target
    return np.array(loss, dtype=np.float32)


# revision 11
# speedup vs baseline: 1.2490x; 1.1358x over previous
# Culled Trainium2 Bass kernel for nn_DoseOptimizationLoss (v3).
#
# Same numeric core as v1 (fp16 hi/lo-split K=7/seed matmuls -> wide Exp ->
# strided-AP seed reduce -> masked-sigmoid epilogue), plus input-dependent
# culling:
#   * voxel rows regrouped into 16x8 spatial (i,j) blocks (128 rows = 1 tile)
#     so each seed's Gaussian support touches few tiles;
#   * per (block, seed): survives iff min_k quad <= CUT; non-survivors are
#     skipped entirely (exp(-quad) < e^-CUT contributes nothing);
#   * blocks LPT-assigned to cores to balance surviving work; each core's
#     blocks sorted by item count so the per-slot max across cores (the SPMD
#     program must be identical on all cores) wastes little;
#   * group slots padded with dummy seeds whose C0hi row is huge -> exp -> 0.
# The Bass program depends on x only through the 25 per-slot group counts,
# which are cached; tables/shards are per-input data.

import numpy as np

import concourse.bass as bass
import concourse.bacc as bacc
import concourse.mybir as mybir
import concourse.tile as tile
from contextlib import ExitStack

VOL = 160
S = 32
SIGMA = np.array([8.0, 4.0, 4.0])
N_CORES = 8
BI, BJ = 16, 8              # spatial block = 128 rows = one tile
NBLK = (VOL // BI) * (VOL // BJ)       # 200 blocks
TILES = NBLK // N_CORES                # 25 per core
KT = 21                     # matmul contraction rows (7 per seed slot)
NCOL = 512                  # moving cols per matmul = one PSUM bank
PAD_BIG = 60000.0
CUT = 14.0                  # drop (block, seed) with min quad > CUT
DVH_RATE = 0.9
F32 = mybir.dt.float32
F16 = mybir.dt.float16


def _seed_params(x):
    """float64 port of the reference's seed math: centers [S,3], A [S,3,3]."""
    xs = np.asarray(x, dtype=np.float64).reshape(S, 6)
    centers = xs[:, :3] * VOL
    d = xs[:, 3:]
    dot = d[:, 0]
    dot_c = np.clip(dot, -0.999999, 0.999999)
    angle = np.arccos(dot_c)
    z = np.zeros(S)
    axis_raw = np.stack([z, -d[:, 2], d[:, 1]], -1)
    nrm = np.linalg.norm(axis_raw, axis=-1, keepdims=True)
    axis = axis_raw / np.where(nrm > 1e-8, nrm, 1.0)
    cos_t = np.cos(angle)[:, None, None]
    sin_t = np.sin(angle)[:, None, None]
    a0, a1, a2 = axis[:, 0], axis[:, 1], axis[:, 2]
    K = np.stack([np.stack([z, -a2, a1], -1),
                  np.stack([a2, z, -a0], -1),
                  np.stack([-a1, a0, z], -1)], 1)
    eye = np.eye(3)
    R = cos_t * eye + (1.0 - cos_t) * axis[:, :, None] * axis[:, None, :] + sin_t * K
    R = np.where((np.abs(dot) >= 0.99)[:, None, None], eye, R)
    D = np.diag(1.0 / (2.0 * SIGMA ** 2))
    A = np.einsum('ski,kl,slj->sij', R, D, R)
    return centers, A


def _split16(v):
    hi = np.asarray(v, np.float64).astype(np.float16)
    lo = (np.asarray(v, np.float64) - hi.astype(np.float64)).astype(np.float16)
    return hi, lo


def _block_rows():
    """[NBLK, 128] global row ids (row = i*VOL + j) for each spatial block."""
    i = np.arange(VOL)
    j = np.arange(VOL)
    I, J = np.meshgrid(i, j, indexing='ij')
    rows = (I * VOL + J).reshape(VOL // BI, BI, VOL // BJ, BJ)
    return rows.transpose(0, 2, 1, 3).reshape(NBLK, 128)


def _row_coeffs(centers, A):
    """C0, Q [S, VOL*VOL] and alpha [S] for all rows, plus per-row min quad."""
    rows = np.arange(VOL * VOL)
    i = (rows // VOL).astype(np.float64)
    j = (rows % VOL).astype(np.float64)
    d0 = i[None, :] - centers[:, 0:1]
    d1 = j[None, :] - centers[:, 1:2]
    c2 = centers[:, 2:3]
    a00 = A[:, 0, 0:1]; a01 = A[:, 0, 1:2]; a02 = A[:, 0, 2:3]
    a11 = A[:, 1, 1:2]; a12 = A[:, 1, 2:3]; a22 = A[:, 2, 2:3]
    lin = a02 * d0 + a12 * d1
    Q = 2.0 * lin - 2.0 * a22 * c2
    C0 = a00 * d0 * d0 + 2.0 * a01 * d0 * d1 + a11 * d1 * d1 \
        - 2.0 * lin * c2 + a22 * c2 * c2
    alpha = a22[:, 0]
    mu = -Q / (2.0 * alpha[:, None])
    beta = C0 - Q * Q / (4.0 * alpha[:, None])
    qmin = np.where(mu < 0.0, C0,
                    np.where(mu > VOL - 1.0,
                             C0 + Q * (VOL - 1.0) + alpha[:, None] * (VOL - 1.0) ** 2,
                             beta))
    return C0, Q, alpha, qmin


def plan(x):
    """Input-dependent schedule.

    Returns dict with:
      blocks_of_core [N_CORES, TILES] block ids (slot order),
      seeds_of      {(core, slot): [seed ids]},
      G             [TILES] group count per slot (same for all cores),
    """
    centers, A = _seed_params(x)
    C0, Q, alpha, qmin = _row_coeffs(centers, A)
    brows = _block_rows()                       # [NBLK, 128]
    # surviving seeds per block
    qmin_blk = qmin[:, brows].min(axis=2)       # [S, NBLK]
    alive = qmin_blk <= CUT                     # [S, NBLK]
    counts = alive.sum(axis=0)                  # [NBLK]

    # LPT assignment: 25 blocks per core, balancing total item count
    order = np.argsort(-counts, kind='stable')
    loads = np.zeros(N_CORES, dtype=int)
    nblk = np.zeros(N_CORES, dtype=int)
    blocks_of_core = [[] for _ in range(N_CORES)]
    for b in order:
        elig = [c for c in range(N_CORES) if nblk[c] < TILES]
        c = min(elig, key=lambda c: (loads[c], c))
        blocks_of_core[c].append(int(b))
        loads[c] += int(counts[b])
        nblk[c] += 1
    # slot order: per core, descending item count
    for c in range(N_CORES):
        blocks_of_core[c].sort(key=lambda b: -int(counts[b]))
    # per-slot group counts (max over cores)
    G = []
    for t in range(TILES):
        m = max(int(counts[blocks_of_core[c][t]]) for c in range(N_CORES))
        G.append((m + 2) // 3)
    seeds_of = {}
    for c in range(N_CORES):
        for t in range(TILES):
            b = blocks_of_core[c][t]
            seeds_of[(c, t)] = [int(s) for s in np.nonzero(alive[:, b])[0]]
    return {
        "blocks_of_core": np.array([bc for bc in blocks_of_core]),
        "seeds_of": seeds_of,
        "G": G,
        "centers": centers,
        "A": A,
    }


def _rhs_table():
    """Single k-pattern block [KT, NCOL] shared by every group.

    Columns are (k, s)-interleaved: col = 3*k + s for k in [0,160), seed
    slot s in [0,3) — so the seed fold in the reduce reads contiguously.
    """
    k = np.arange(VOL, dtype=np.float64)
    k2hi, k2lo = _split16(k * k)
    rhs = np.zeros((KT, NCOL), np.float16)
    for sl in range(3):
        r = 7 * sl
        cols = slice(sl, 480, 3)
        rhs[r + 0, cols] = np.float16(1.0)
        rhs[r + 1, cols] = np.float16(1.0)
        rhs[r + 2, cols] = k.astype(np.float16)
        rhs[r + 3, cols] = k.astype(np.float16)
        rhs[r + 4, cols] = k2hi
        rhs[r + 5, cols] = k2lo
        rhs[r + 6, cols] = k2hi
    rhs[0, 480:NCOL] = np.float16(PAD_BIG)  # pad cols via slot0 C0hi row
    return rhs


def _lhs_table(pl, core):
    """[KT, GT*128] fp16 coefficient matrix for one core (GT = sum(G))."""
    centers, A = pl["centers"], pl["A"]
    G = pl["G"]
    GT = sum(G)
    brows = _block_rows()
    C0, Q, alpha, _ = _row_coeffs(centers, A)
    C0hi, C0lo = _split16(C0)
    Qhi, Qlo = _split16(Q)
    Ahi, Alo = _split16(alpha)

    lhs = np.zeros((KT, max(GT, 1) * 128), np.float16)
    if GT == 0:
        lhs[0, :] = np.float16(PAD_BIG)
    gidx = 0
    for t in range(TILES):
        b = pl["blocks_of_core"][core][t]
        rows = brows[b]                        # [128] global row ids
        seeds = pl["seeds_of"][(core, t)]
        for g in range(G[t]):
            base = gidx * 128
            for sl in range(3):
                si = g * 3 + sl
                r = 7 * sl
                if si < len(seeds):
                    s = seeds[si]
                    lhs[r + 0, base:base + 128] = C0hi[s, rows]
                    lhs[r + 1, base:base + 128] = C0lo[s, rows]
                    lhs[r + 2, base:base + 128] = Qhi[s, rows]
                    lhs[r + 3, base:base + 128] = Qlo[s, rows]
                    lhs[r + 4, base:base + 128] = Ahi[s]
                    lhs[r + 5, base:base + 128] = Ahi[s]
                    lhs[r + 6, base:base + 128] = Alo[s]
                else:
                    lhs[r + 0, base:base + 128] = np.float16(PAD_BIG)
            gidx += 1
    return lhs


def _build_nc(G):
    """Build the SPMD program for per-slot group counts G (len TILES)."""
    GT = sum(G)
    nc = bacc.Bacc("TRN2", target_bir_lowering=False, debug=False,
                   num_devices=N_CORES)
    rv = nc.declare_dram_parameter("rv", [TILES * 128, VOL], F32, isOutput=False)
    om = nc.declare_dram_parameter("om", [TILES * 128, VOL], F32, isOutput=False)
    lhs = nc.declare_dram_parameter("lhs", [KT, max(GT, 1) * 128], F16,
                                    isOutput=False)
    rhs = nc.declare_dram_parameter("rhs", [KT, NCOL], F16, isOutput=False)
    partials = nc.declare_dram_parameter("partials", [1, 4], F32, isOutput=True)

    add = mybir.AluOpType.add
    mult = mybir.AluOpType.mult
    Exp = mybir.ActivationFunctionType.Exp
    Sigmoid = mybir.ActivationFunctionType.Sigmoid

    # (tile, local group) stream in program order, packed into PSUM fills of 4
    work = [(t, g) for t in range(TILES) for g in range(G[t])]

    with ExitStack() as ctx:
        tc = ctx.enter_context(tile.TileContext(nc))
        cpool = ctx.enter_context(tc.tile_pool(name="const", bufs=1))
        ppool = ctx.enter_context(tc.tile_pool(name="psum", bufs=2, space="PSUM"))
        gpool = ctx.enter_context(tc.tile_pool(name="g", bufs=4))
        tpool = ctx.enter_context(tc.tile_pool(name="tmp", bufs=4))

        lhs_sb = cpool.tile([KT, max(GT, 1) * 128], F16)
        rhs_sb = cpool.tile([KT, NCOL], F16)
        rv_sb = cpool.tile([128, TILES * VOL], F32)
        om_sb = cpool.tile([128, TILES * VOL], F32)
        rad = cpool.tile([128, TILES * VOL], F32)

        nc.sync.dma_start(rhs_sb[:], rhs[:])
        # split the lhs table DMA so early fills aren't gated on the tail
        nsplit = 4
        cols = max(GT, 1) * 128
        step = ((cols + nsplit - 1) // nsplit + 127) // 128 * 128
        for o in range(0, cols, step):
            e = min(o + step, cols)
            nc.sync.dma_start(lhs_sb[:, o:e], lhs[:, o:e])
        nc.sync.dma_start(
            rv_sb[:].rearrange("p (t k) -> p t k", t=TILES),
            rv[:].rearrange("(t p) k -> p t k", p=128))
        nc.sync.dma_start(
            om_sb[:].rearrange("p (t k) -> p t k", t=TILES),
            om[:].rearrange("(t p) k -> p t k", p=128))

        # tiles with no groups: zero their rad slice
        written = set()
        for t in range(TILES):
            if G[t] == 0:
                nc.vector.memset(rad[:, t * VOL:(t + 1) * VOL], 0.0)
                written.add(t)

        # Phase 1: stream groups through PSUM fills of up to 4.  The
        # seed-sum alternates between VectorE (fused 4D strided reduce)
        # and GpSimd (block adds) to split the DVE bottleneck.
        nfills = (len(work) + 3) // 4
        for fi, f0 in enumerate(range(0, len(work), 4)):
            fill = work[f0:f0 + 4]
            n = len(fill)
            q = ppool.tile([128, 4 * NCOL], F32, tag="q")
            for gl, (t, g) in enumerate(fill):
                gi = f0 + gl   # global group index = lhs slot
                nc.tensor.matmul(
                    q[:, gl * NCOL:(gl + 1) * NCOL],
                    lhsT=lhs_sb[:, gi * 128:(gi + 1) * 128],
                    rhs=rhs_sb[:],
                    start=True, stop=True)
            # Exp skips the 32 pad cols of each 512 block: strided PSUM
            # read, packed 480-wide SBUF write.
            gt = gpool.tile([128, 4 * 480], F32, tag="g")
            q_src = (q[:, :n * NCOL]
                     .rearrange("p (g c) -> p g c", g=n)[:, :, 0:480])
            nc.scalar.activation(
                gt[:, :n * 480].rearrange("p (g c) -> p g c", g=n),
                q_src, Exp, scale=-1.0)
            use_gpsimd = (fi % 5 in (1, 3))
            # accumulate per contiguous same-tile run within the fill
            i0 = 0
            while i0 < n:
                t = fill[i0][0]
                i1 = i0
                while i1 < n and fill[i1][0] == t:
                    i1 += 1
                m = i1 - i0
                rad_t = rad[:, t * VOL:(t + 1) * VOL]
                if not use_gpsimd:
                    # cols of each 480-chunk are (k, s) interleaved
                    red_src = (gt[:, i0 * 480:i1 * 480]
                               .rearrange("p (g k s) -> p k g s", g=m, s=3))
                    if t not in written:
                        nc.vector.tensor_reduce(
                            rad_t, red_src, axis=mybir.AxisListType.XY, op=add)
                        written.add(t)
                    else:
                        tmp = tpool.tile([128, VOL], F32, tag="tmp")
                        nc.vector.tensor_reduce(
                            tmp[:], red_src, axis=mybir.AxisListType.XY, op=add)
                        nc.vector.tensor_add(rad_t, rad_t, tmp[:])
                else:
                    # GpSimd: wide adds over the 480-chunks, then a
                    # strided (k,s) fold
                    eng = nc.gpsimd
                    src = gt[:, i0 * 480:i1 * 480]
                    if m == 1:
                        ssum = src[:, 0:480]
                    elif m == 2:
                        bt = tpool.tile([128, 480], F32, tag="gb")
                        eng.tensor_add(bt[:], src[:, 0:480], src[:, 480:960])
                        ssum = bt[:]
                    elif m == 3:
                        bt = tpool.tile([128, 480], F32, tag="gb")
                        eng.tensor_add(bt[:], src[:, 0:480], src[:, 960:1440])
                        bt2 = tpool.tile([128, 480], F32, tag="gb2")
                        eng.tensor_add(bt2[:], bt[:], src[:, 480:960])
                        ssum = bt2[:]
                    else:  # m == 4
                        bt = tpool.tile([128, 960], F32, tag="gbw")
                        eng.tensor_add(bt[:], src[:, 0:960], src[:, 960:1920])
                        bt2 = tpool.tile([128, 480], F32, tag="gb2")
                        eng.tensor_add(bt2[:], bt[:, 0:480], bt[:, 480:960])
                        ssum = bt2[:]
                    ks = ssum.rearrange("p (k s) -> p k s", s=3)
                    u = tpool.tile([128, VOL], F32, tag="gu")
                    eng.tensor_add(u[:], ks[:, :, 0], ks[:, :, 1])
                    if t not in written:
                        eng.tensor_add(rad_t, u[:], ks[:, :, 2])
                        written.add(t)
                    else:
                        u2 = tpool.tile([128, VOL], F32, tag="gu2")
                        eng.tensor_add(u2[:], u[:], ks[:, :, 2])
                        eng.tensor_add(rad_t, rad_t, u2[:])
                i0 = i1

        # Phase 2: masked sums
        CH = 2
        CW = TILES * VOL // CH
        eparts = cpool.tile([128, CH], F32)
        tparts = cpool.tile([128, CH], F32)
        euparts = cpool.tile([128, CH], F32)
        uparts = cpool.tile([128, CH], F32)
        acc4 = cpool.tile([128, 4], F32)
        bneg1 = cpool.tile([128, 1], F32)
        bneg50 = cpool.tile([128, 1], F32)
        nc.vector.memset(bneg1[:], -1.0)
        nc.vector.memset(bneg50[:], -50.0)
        for c in range(CH):
            sl = slice(c * CW, (c + 1) * CW)
            eff = tpool.tile([128, CW], F32, tag="eff")
            nc.vector.scalar_tensor_tensor(
                eff[:], rad[:, sl], 1.0, rv_sb[:, sl], mult, mult,
                accum_out=eparts[:, c:c + 1])
            nc.scalar.activation(eff[:], eff[:], Sigmoid, bias=bneg1[:],
                                 scale=1.0, accum_out=euparts[:, c:c + 1])
            outv = tpool.tile([128, CW], F32, tag="outv")
            nc.vector.tensor_mul(outv[:], rad[:, sl], om_sb[:, sl])
            nc.scalar.activation(outv[:], outv[:], Sigmoid, bias=bneg50[:],
                                 scale=100.0, accum_out=uparts[:, c:c + 1])
            nc.vector.tensor_reduce(tparts[:, c:c + 1], rad[:, sl],
                                    axis=mybir.AxisListType.X, op=add)
        for idx, p4 in enumerate([eparts, tparts, euparts, uparts]):
            nc.vector.tensor_reduce(acc4[:, idx:idx + 1], p4[:],
                                    axis=mybir.AxisListType.X, op=add)
        ones = cpool.tile([128, 1], F32)
        nc.vector.memset(ones[:], 1.0)
        accq = ppool.tile([1, 4], F32, tag="q")
        nc.tensor.matmul(accq[:], lhsT=ones[:], rhs=acc4[:], start=True,
                         stop=True)
        res = tpool.tile([1, 4], F32, tag="res")
        nc.scalar.copy(res[:], accq[:])
        nc.sync.dma_start(partials[:], res[:])
    nc.compile()
    return nc


_NC_CACHE = {}
LAST_RESULT = None


def _get_nc(G):
    key = tuple(G)
    if key not in _NC_CACHE:
        _NC_CACHE[key] = _build_nc(list(G))
    return _NC_CACHE[key]


def kernel(x, radiation_volume, outside_mask):
    from concourse.bass_utils import run_bass_kernel_spmd

    pl = plan(x)
    nc = _get_nc(pl["G"])
    rv2 = np.asarray(radiation_volume, np.float32).reshape(VOL * VOL, VOL)
    om2 = np.asarray(outside_mask, np.float32).reshape(VOL * VOL, VOL)
    brows = _block_rows()
    rhs = _rhs_table()
    in_maps = []
    for c in range(N_CORES):
        rows = brows[pl["blocks_of_core"][c]].reshape(-1)   # [3200]
        in_maps.append({
            "rv": np.ascontiguousarray(rv2[rows]),
            "om": np.ascontiguousarray(om2[rows]),
            "lhs": np.ascontiguousarray(_lhs_table(pl, c)),
            "rhs": rhs,
        })
    out = run_bass_kernel_spmd(nc, in_maps, list(range(N_CORES)))
    global LAST_RESULT
    LAST_RESULT = out
    parts = np.stack([out.results[i]["partials"][0] for i in range(N_CORES)])
    E, T, EU, U = parts.sum(axis=0, dtype=np.float64)
    num_target = float(np.sum(rv2, dtype=np.float64))
    loss = (DVH_RATE - EU / num_target) + (1.0 - E / T) + U / num_target
    return np.array(loss, dtype=np.float32)


# revision 12
# speedup vs baseline: 1.2840x; 1.0280x over previous
# Culled Trainium2 Bass kernel for nn_DoseOptimizationLoss (v3).
#
# Same numeric core as v1 (fp16 hi/lo-split K=7/seed matmuls -> wide Exp ->
# strided-AP seed reduce -> masked-sigmoid epilogue), plus input-dependent
# culling:
#   * voxel rows regrouped into 16x8 spatial (i,j) blocks (128 rows = 1 tile)
#     so each seed's Gaussian support touches few tiles;
#   * per (block, seed): survives iff min_k quad <= CUT; non-survivors are
#     skipped entirely (exp(-quad) < e^-CUT contributes nothing);
#   * blocks LPT-assigned to cores to balance surviving work; each core's
#     blocks sorted by item count so the per-slot max across cores (the SPMD
#     program must be identical on all cores) wastes little;
#   * group slots padded with dummy seeds whose C0hi row is huge -> exp -> 0.
# The Bass program depends on x only through the 25 per-slot group counts,
# which are cached; tables/shards are per-input data.

import numpy as np

import concourse.bass as bass
import concourse.bacc as bacc
import concourse.mybir as mybir
import concourse.tile as tile
from contextlib import ExitStack

VOL = 160
S = 32
SIGMA = np.array([8.0, 4.0, 4.0])
N_CORES = 8
BI, BJ = 16, 8              # spatial block = 128 rows = one tile
NBLK = (VOL // BI) * (VOL // BJ)       # 200 blocks
TILES = NBLK // N_CORES                # 25 per core
KT = 21                     # matmul contraction rows (7 per seed slot)
NCOL = 512                  # moving cols per matmul = one PSUM bank
PAD_BIG = 60000.0
CUT = 10.0                  # drop (block, seed) with min quad > CUT
DVH_RATE = 0.9
F32 = mybir.dt.float32
F16 = mybir.dt.float16


def _seed_params(x):
    """float64 port of the reference's seed math: centers [S,3], A [S,3,3]."""
    xs = np.asarray(x, dtype=np.float64).reshape(S, 6)
    centers = xs[:, :3] * VOL
    d = xs[:, 3:]
    dot = d[:, 0]
    dot_c = np.clip(dot, -0.999999, 0.999999)
    angle = np.arccos(dot_c)
    z = np.zeros(S)
    axis_raw = np.stack([z, -d[:, 2], d[:, 1]], -1)
    nrm = np.linalg.norm(axis_raw, axis=-1, keepdims=True)
    axis = axis_raw / np.where(nrm > 1e-8, nrm, 1.0)
    cos_t = np.cos(angle)[:, None, None]
    sin_t = np.sin(angle)[:, None, None]
    a0, a1, a2 = axis[:, 0], axis[:, 1], axis[:, 2]
    K = np.stack([np.stack([z, -a2, a1], -1),
                  np.stack([a2, z, -a0], -1),
                  np.stack([-a1, a0, z], -1)], 1)
    eye = np.eye(3)
    R = cos_t * eye + (1.0 - cos_t) * axis[:, :, None] * axis[:, None, :] + sin_t * K
    R = np.where((np.abs(dot) >= 0.99)[:, None, None], eye, R)
    D = np.diag(1.0 / (2.0 * SIGMA ** 2))
    A = np.einsum('ski,kl,slj->sij', R, D, R)
    return centers, A


def _split16(v):
    hi = np.asarray(v, np.float64).astype(np.float16)
    lo = (np.asarray(v, np.float64) - hi.astype(np.float64)).astype(np.float16)
    return hi, lo


def _block_rows():
    """[NBLK, 128] global row ids (row = i*VOL + j) for each spatial block."""
    i = np.arange(VOL)
    j = np.arange(VOL)
    I, J = np.meshgrid(i, j, indexing='ij')
    rows = (I * VOL + J).reshape(VOL // BI, BI, VOL // BJ, BJ)
    return rows.transpose(0, 2, 1, 3).reshape(NBLK, 128)


def _row_coeffs(centers, A):
    """C0, Q [S, VOL*VOL] and alpha [S] for all rows, plus per-row min quad."""
    rows = np.arange(VOL * VOL)
    i = (rows // VOL).astype(np.float64)
    j = (rows % VOL).astype(np.float64)
    d0 = i[None, :] - centers[:, 0:1]
    d1 = j[None, :] - centers[:, 1:2]
    c2 = centers[:, 2:3]
    a00 = A[:, 0, 0:1]; a01 = A[:, 0, 1:2]; a02 = A[:, 0, 2:3]
    a11 = A[:, 1, 1:2]; a12 = A[:, 1, 2:3]; a22 = A[:, 2, 2:3]
    lin = a02 * d0 + a12 * d1
    Q = 2.0 * lin - 2.0 * a22 * c2
    C0 = a00 * d0 * d0 + 2.0 * a01 * d0 * d1 + a11 * d1 * d1 \
        - 2.0 * lin * c2 + a22 * c2 * c2
    alpha = a22[:, 0]
    mu = -Q / (2.0 * alpha[:, None])
    beta = C0 - Q * Q / (4.0 * alpha[:, None])
    qmin = np.where(mu < 0.0, C0,
                    np.where(mu > VOL - 1.0,
                             C0 + Q * (VOL - 1.0) + alpha[:, None] * (VOL - 1.0) ** 2,
                             beta))
    return C0, Q, alpha, qmin


def plan(x):
    """Input-dependent schedule.

    Returns dict with:
      blocks_of_core [N_CORES, TILES] block ids (slot order),
      seeds_of      {(core, slot): [seed ids]},
      G             [TILES] group count per slot (same for all cores),
    """
    centers, A = _seed_params(x)
    C0, Q, alpha, qmin = _row_coeffs(centers, A)
    brows = _block_rows()                       # [NBLK, 128]
    # surviving seeds per block
    qmin_blk = qmin[:, brows].min(axis=2)       # [S, NBLK]
    alive = qmin_blk <= CUT                     # [S, NBLK]
    counts = alive.sum(axis=0)                  # [NBLK]

    # LPT assignment: 25 blocks per core, balancing total item count
    order = np.argsort(-counts, kind='stable')
    loads = np.zeros(N_CORES, dtype=int)
    nblk = np.zeros(N_CORES, dtype=int)
    blocks_of_core = [[] for _ in range(N_CORES)]
    for b in order:
        elig = [c for c in range(N_CORES) if nblk[c] < TILES]
        c = min(elig, key=lambda c: (loads[c], c))
        blocks_of_core[c].append(int(b))
        loads[c] += int(counts[b])
        nblk[c] += 1
    # slot order: per core, descending item count
    for c in range(N_CORES):
        blocks_of_core[c].sort(key=lambda b: -int(counts[b]))
    # per-slot group counts (max over cores)
    G = []
    for t in range(TILES):
        m = max(int(counts[blocks_of_core[c][t]]) for c in range(N_CORES))
        G.append((m + 2) // 3)
    seeds_of = {}
    for c in range(N_CORES):
        for t in range(TILES):
            b = blocks_of_core[c][t]
            seeds_of[(c, t)] = [int(s) for s in np.nonzero(alive[:, b])[0]]
    return {
        "blocks_of_core": np.array([bc for bc in blocks_of_core]),
        "seeds_of": seeds_of,
        "G": G,
        "centers": centers,
        "A": A,
    }


def _rhs_table():
    """Single k-pattern block [KT, NCOL] shared by every group.

    Columns are (k, s)-interleaved: col = 3*k + s for k in [0,160), seed
    slot s in [0,3) — so the seed fold in the reduce reads contiguously.
    """
    k = np.arange(VOL, dtype=np.float64)
    k2hi, k2lo = _split16(k * k)
    rhs = np.zeros((KT, NCOL), np.float16)
    for sl in range(3):
        r = 7 * sl
        cols = slice(sl, 480, 3)
        rhs[r + 0, cols] = np.float16(1.0)
        rhs[r + 1, cols] = np.float16(1.0)
        rhs[r + 2, cols] = k.astype(np.float16)
        rhs[r + 3, cols] = k.astype(np.float16)
        rhs[r + 4, cols] = k2hi
        rhs[r + 5, cols] = k2lo
        rhs[r + 6, cols] = k2hi
    rhs[0, 480:NCOL] = np.float16(PAD_BIG)  # pad cols via slot0 C0hi row
    return rhs


def _lhs_table(pl, core):
    """[KT, GT*128] fp16 coefficient matrix for one core (GT = sum(G))."""
    centers, A = pl["centers"], pl["A"]
    G = pl["G"]
    GT = sum(G)
    brows = _block_rows()
    C0, Q, alpha, _ = _row_coeffs(centers, A)
    C0hi, C0lo = _split16(C0)
    Qhi, Qlo = _split16(Q)
    Ahi, Alo = _split16(alpha)

    lhs = np.zeros((KT, max(GT, 1) * 128), np.float16)
    if GT == 0:
        lhs[0, :] = np.float16(PAD_BIG)
    gidx = 0
    for t in range(TILES):
        b = pl["blocks_of_core"][core][t]
        rows = brows[b]                        # [128] global row ids
        seeds = pl["seeds_of"][(core, t)]
        for g in range(G[t]):
            base = gidx * 128
            for sl in range(3):
                si = g * 3 + sl
                r = 7 * sl
                if si < len(seeds):
                    s = seeds[si]
                    lhs[r + 0, base:base + 128] = C0hi[s, rows]
                    lhs[r + 1, base:base + 128] = C0lo[s, rows]
                    lhs[r + 2, base:base + 128] = Qhi[s, rows]
                    lhs[r + 3, base:base + 128] = Qlo[s, rows]
                    lhs[r + 4, base:base + 128] = Ahi[s]
                    lhs[r + 5, base:base + 128] = Ahi[s]
                    lhs[r + 6, base:base + 128] = Alo[s]
                else:
                    lhs[r + 0, base:base + 128] = np.float16(PAD_BIG)
            gidx += 1
    return lhs


def _build_nc(G):
    """Build the SPMD program for per-slot group counts G (len TILES)."""
    GT = sum(G)
    nc = bacc.Bacc("TRN2", target_bir_lowering=False, debug=False,
                   num_devices=N_CORES)
    rv = nc.declare_dram_parameter("rv", [TILES * 128, VOL], F32, isOutput=False)
    om = nc.declare_dram_parameter("om", [TILES * 128, VOL], F32, isOutput=False)
    lhs = nc.declare_dram_parameter("lhs", [KT, max(GT, 1) * 128], F16,
                                    isOutput=False)
    rhs = nc.declare_dram_parameter("rhs", [KT, NCOL], F16, isOutput=False)
    partials = nc.declare_dram_parameter("partials", [1, 4], F32, isOutput=True)

    add = mybir.AluOpType.add
    mult = mybir.AluOpType.mult
    Exp = mybir.ActivationFunctionType.Exp
    Sigmoid = mybir.ActivationFunctionType.Sigmoid

    # (tile, local group) stream in program order, packed into PSUM fills of 4
    work = [(t, g) for t in range(TILES) for g in range(G[t])]

    with ExitStack() as ctx:
        tc = ctx.enter_context(tile.TileContext(nc))
        cpool = ctx.enter_context(tc.tile_pool(name="const", bufs=1))
        ppool = ctx.enter_context(tc.tile_pool(name="psum", bufs=2, space="PSUM"))
        gpool = ctx.enter_context(tc.tile_pool(name="g", bufs=4))
        tpool = ctx.enter_context(tc.tile_pool(name="tmp", bufs=4))

        lhs_sb = cpool.tile([KT, max(GT, 1) * 128], F16)
        rhs_sb = cpool.tile([KT, NCOL], F16)
        rv_sb = cpool.tile([128, TILES * VOL], F32)
        om_sb = cpool.tile([128, TILES * VOL], F32)
        rad = cpool.tile([128, TILES * VOL], F32)

        nc.sync.dma_start(rhs_sb[:], rhs[:])
        # split the lhs table DMA so early fills aren't gated on the tail:
        # a small head chunk (first two fills) then three even chunks
        cols = max(GT, 1) * 128
        head = min(8 * 128, cols)
        bounds = [0, head]
        rest = cols - head
        for p in (1, 2, 3):
            bounds.append(head + (rest * p // 3) // 128 * 128 if p < 3 else cols)
        for o, e in zip(bounds[:-1], bounds[1:]):
            if e > o:
                nc.sync.dma_start(lhs_sb[:, o:e], lhs[:, o:e])
        nc.sync.dma_start(
            rv_sb[:].rearrange("p (t k) -> p t k", t=TILES),
            rv[:].rearrange("(t p) k -> p t k", p=128))
        nc.sync.dma_start(
            om_sb[:].rearrange("p (t k) -> p t k", t=TILES),
            om[:].rearrange("(t p) k -> p t k", p=128))

        # tiles with no groups: zero their rad slice
        written = set()
        for t in range(TILES):
            if G[t] == 0:
                nc.vector.memset(rad[:, t * VOL:(t + 1) * VOL], 0.0)
                written.add(t)

        # Phase 1: stream groups through PSUM fills of up to 4.  The
        # seed-sum alternates between VectorE (fused 4D strided reduce)
        # and GpSimd (block adds) to split the DVE bottleneck.
        nfills = (len(work) + 3) // 4
        for fi, f0 in enumerate(range(0, len(work), 4)):
            fill = work[f0:f0 + 4]
            n = len(fill)
            q = ppool.tile([128, 4 * NCOL], F32, tag="q")
            for gl, (t, g) in enumerate(fill):
                gi = f0 + gl   # global group index = lhs slot
                nc.tensor.matmul(
                    q[:, gl * NCOL:(gl + 1) * NCOL],
                    lhsT=lhs_sb[:, gi * 128:(gi + 1) * 128],
                    rhs=rhs_sb[:],
                    start=True, stop=True)
            # Exp skips the 32 pad cols of each 512 block: strided PSUM
            # read, packed 480-wide SBUF write.
            gt = gpool.tile([128, 4 * 480], F32, tag="g")
            q_src = (q[:, :n * NCOL]
                     .rearrange("p (g c) -> p g c", g=n)[:, :, 0:480])
            nc.scalar.activation(
                gt[:, :n * 480].rearrange("p (g c) -> p g c", g=n),
                q_src, Exp, scale=-1.0)
            use_gpsimd = (fi % 5 in (1, 3))
            # accumulate per contiguous same-tile run within the fill
            i0 = 0
            while i0 < n:
                t = fill[i0][0]
                i1 = i0
                while i1 < n and fill[i1][0] == t:
                    i1 += 1
                m = i1 - i0
                rad_t = rad[:, t * VOL:(t + 1) * VOL]
                if not use_gpsimd:
                    # cols of each 480-chunk are (k, s) interleaved
                    red_src = (gt[:, i0 * 480:i1 * 480]
                               .rearrange("p (g k s) -> p k g s", g=m, s=3))
                    if t not in written:
                        nc.vector.tensor_reduce(
                            rad_t, red_src, axis=mybir.AxisListType.XY, op=add)
                        written.add(t)
                    else:
                        tmp = tpool.tile([128, VOL], F32, tag="tmp")
                        nc.vector.tensor_reduce(
                            tmp[:], red_src, axis=mybir.AxisListType.XY, op=add)
                        nc.vector.tensor_add(rad_t, rad_t, tmp[:])
                else:
                    # GpSimd: wide adds over the 480-chunks, then a
                    # strided (k,s) fold
                    eng = nc.gpsimd
                    src = gt[:, i0 * 480:i1 * 480]
                    if m == 1:
                        ssum = src[:, 0:480]
                    elif m == 2:
                        bt = tpool.tile([128, 480], F32, tag="gb")
                        eng.tensor_add(bt[:], src[:, 0:480], src[:, 480:960])
                        ssum = bt[:]
                    elif m == 3:
                        bt = tpool.tile([128, 480], F32, tag="gb")
                        eng.tensor_add(bt[:], src[:, 0:480], src[:, 960:1440])
                        bt2 = tpool.tile([128, 480], F32, tag="gb2")
                        eng.tensor_add(bt2[:], bt[:], src[:, 480:960])
                        ssum = bt2[:]
                    else:  # m == 4
                        bt = tpool.tile([128, 960], F32, tag="gbw")
                        eng.tensor_add(bt[:], src[:, 0:960], src[:, 960:1920])
                        bt2 = tpool.tile([128, 480], F32, tag="gb2")
                        eng.tensor_add(bt2[:], bt[:, 0:480], bt[:, 480:960])
                        ssum = bt2[:]
                    ks = ssum.rearrange("p (k s) -> p k s", s=3)
                    u = tpool.tile([128, VOL], F32, tag="gu")
                    eng.tensor_add(u[:], ks[:, :, 0], ks[:, :, 1])
                    if t not in written:
                        eng.tensor_add(rad_t, u[:], ks[:, :, 2])
                        written.add(t)
                    else:
                        u2 = tpool.tile([128, VOL], F32, tag="gu2")
                        eng.tensor_add(u2[:], u[:], ks[:, :, 2])
                        eng.tensor_add(rad_t, rad_t, u2[:])
                i0 = i1

        # Phase 2: masked sums
        CH = 2
        CW = TILES * VOL // CH
        eparts = cpool.tile([128, CH], F32)
        tparts = cpool.tile([128, CH], F32)
        euparts = cpool.tile([128, CH], F32)
        uparts = cpool.tile([128, CH], F32)
        acc4 = cpool.tile([128, 4], F32)
        bneg1 = cpool.tile([128, 1], F32)
        bneg50 = cpool.tile([128, 1], F32)
        nc.vector.memset(bneg1[:], -1.0)
        nc.vector.memset(bneg50[:], -50.0)
        for c in range(CH):
            sl = slice(c * CW, (c + 1) * CW)
            eff = tpool.tile([128, CW], F32, tag="eff")
            nc.vector.scalar_tensor_tensor(
                eff[:], rad[:, sl], 1.0, rv_sb[:, sl], mult, mult,
                accum_out=eparts[:, c:c + 1])
            nc.scalar.activation(eff[:], eff[:], Sigmoid, bias=bneg1[:],
                                 scale=1.0, accum_out=euparts[:, c:c + 1])
            outv = tpool.tile([128, CW], F32, tag="outv")
            nc.vector.tensor_mul(outv[:], rad[:, sl], om_sb[:, sl])
            nc.scalar.activation(outv[:], outv[:], Sigmoid, bias=bneg50[:],
                                 scale=100.0, accum_out=uparts[:, c:c + 1])
            nc.vector.tensor_reduce(tparts[:, c:c + 1], rad[:, sl],
                                    axis=mybir.AxisListType.X, op=add)
        for idx, p4 in enumerate([eparts, tparts, euparts, uparts]):
            nc.vector.tensor_reduce(acc4[:, idx:idx + 1], p4[:],
                                    axis=mybir.AxisListType.X, op=add)
        ones = cpool.tile([128, 1], F32)
        nc.vector.memset(ones[:], 1.0)
        accq = ppool.tile([1, 4], F32, tag="q")
        nc.tensor.matmul(accq[:], lhsT=ones[:], rhs=acc4[:], start=True,
                         stop=True)
        res = tpool.tile([1, 4], F32, tag="res")
        nc.scalar.copy(res[:], accq[:])
        nc.sync.dma_start(partials[:], res[:])
    nc.compile()
    return nc


_NC_CACHE = {}
LAST_RESULT = None


def _get_nc(G):
    key = tuple(G)
    if key not in _NC_CACHE:
        _NC_CACHE[key] = _build_nc(list(G))
    return _NC_CACHE[key]


def kernel(x, radiation_volume, outside_mask):
    from concourse.bass_utils import run_bass_kernel_spmd

    pl = plan(x)
    nc = _get_nc(pl["G"])
    rv2 = np.asarray(radiation_volume, np.float32).reshape(VOL * VOL, VOL)
    om2 = np.asarray(outside_mask, np.float32).reshape(VOL * VOL, VOL)
    brows = _block_rows()
    rhs = _rhs_table()
    in_maps = []
    for c in range(N_CORES):
        rows = brows[pl["blocks_of_core"][c]].reshape(-1)   # [3200]
        in_maps.append({
            "rv": np.ascontiguousarray(rv2[rows]),
            "om": np.ascontiguousarray(om2[rows]),
            "lhs": np.ascontiguousarray(_lhs_table(pl, c)),
            "rhs": rhs,
        })
    out = run_bass_kernel_spmd(nc, in_maps, list(range(N_CORES)))
    global LAST_RESULT
    LAST_RESULT = out
    parts = np.stack([out.results[i]["partials"][0] for i in range(N_CORES)])
    E, T, EU, U = parts.sum(axis=0, dtype=np.float64)
    num_target = float(np.sum(rv2, dtype=np.float64))
    loss = (DVH_RATE - EU / num_target) + (1.0 - E / T) + U / num_target
    return np.array(loss, dtype=np.float32)


# revision 13
# speedup vs baseline: 1.4667x; 1.1423x over previous
# Culled Trainium2 Bass kernel for nn_DoseOptimizationLoss (v3).
#
# Same numeric core as v1 (fp16 hi/lo-split K=7/seed matmuls -> wide Exp ->
# strided-AP seed reduce -> masked-sigmoid epilogue), plus input-dependent
# culling:
#   * voxel rows regrouped into 16x8 spatial (i,j) blocks (128 rows = 1 tile)
#     so each seed's Gaussian support touches few tiles;
#   * per (block, seed): survives iff min_k quad <= CUT; non-survivors are
#     skipped entirely (exp(-quad) < e^-CUT contributes nothing);
#   * blocks LPT-assigned to cores to balance surviving work; each core's
#     blocks sorted by item count so the per-slot max across cores (the SPMD
#     program must be identical on all cores) wastes little;
#   * group slots padded with dummy seeds whose C0hi row is huge -> exp -> 0.
# The Bass program depends on x only through the 25 per-slot group counts,
# which are cached; tables/shards are per-input data.

import numpy as np

import concourse.bass as bass
import concourse.bacc as bacc
import concourse.mybir as mybir
import concourse.tile as tile
from contextlib import ExitStack

VOL = 160
S = 32
SIGMA = np.array([8.0, 4.0, 4.0])
N_CORES = 8
BI, BJ = 16, 8              # spatial block = 128 rows = one tile
NBLK = (VOL // BI) * (VOL // BJ)       # 200 blocks
TILES = NBLK // N_CORES                # 25 per core
KT = 21                     # matmul contraction rows (7 per seed slot)
NCOL = 512                  # moving cols per matmul = one PSUM bank
PAD_BIG = 60000.0
CUT = 8.0                  # drop (block, seed) with min quad > CUT
DVH_RATE = 0.9
F32 = mybir.dt.float32
F16 = mybir.dt.float16


def _seed_params(x):
    """float64 port of the reference's seed math: centers [S,3], A [S,3,3]."""
    xs = np.asarray(x, dtype=np.float64).reshape(S, 6)
    centers = xs[:, :3] * VOL
    d = xs[:, 3:]
    dot = d[:, 0]
    dot_c = np.clip(dot, -0.999999, 0.999999)
    angle = np.arccos(dot_c)
    z = np.zeros(S)
    axis_raw = np.stack([z, -d[:, 2], d[:, 1]], -1)
    nrm = np.linalg.norm(axis_raw, axis=-1, keepdims=True)
    axis = axis_raw / np.where(nrm > 1e-8, nrm, 1.0)
    cos_t = np.cos(angle)[:, None, None]
    sin_t = np.sin(angle)[:, None, None]
    a0, a1, a2 = axis[:, 0], axis[:, 1], axis[:, 2]
    K = np.stack([np.stack([z, -a2, a1], -1),
                  np.stack([a2, z, -a0], -1),
                  np.stack([-a1, a0, z], -1)], 1)
    eye = np.eye(3)
    R = cos_t * eye + (1.0 - cos_t) * axis[:, :, None] * axis[:, None, :] + sin_t * K
    R = np.where((np.abs(dot) >= 0.99)[:, None, None], eye, R)
    D = np.diag(1.0 / (2.0 * SIGMA ** 2))
    A = np.einsum('ski,kl,slj->sij', R, D, R)
    return centers, A


def _split16(v):
    hi = np.asarray(v, np.float64).astype(np.float16)
    lo = (np.asarray(v, np.float64) - hi.astype(np.float64)).astype(np.float16)
    return hi, lo


def _block_rows():
    """[NBLK, 128] global row ids (row = i*VOL + j) for each spatial block."""
    i = np.arange(VOL)
    j = np.arange(VOL)
    I, J = np.meshgrid(i, j, indexing='ij')
    rows = (I * VOL + J).reshape(VOL // BI, BI, VOL // BJ, BJ)
    return rows.transpose(0, 2, 1, 3).reshape(NBLK, 128)


def _row_coeffs(centers, A):
    """C0, Q [S, VOL*VOL] and alpha [S] for all rows, plus per-row min quad."""
    rows = np.arange(VOL * VOL)
    i = (rows // VOL).astype(np.float64)
    j = (rows % VOL).astype(np.float64)
    d0 = i[None, :] - centers[:, 0:1]
    d1 = j[None, :] - centers[:, 1:2]
    c2 = centers[:, 2:3]
    a00 = A[:, 0, 0:1]; a01 = A[:, 0, 1:2]; a02 = A[:, 0, 2:3]
    a11 = A[:, 1, 1:2]; a12 = A[:, 1, 2:3]; a22 = A[:, 2, 2:3]
    lin = a02 * d0 + a12 * d1
    Q = 2.0 * lin - 2.0 * a22 * c2
    C0 = a00 * d0 * d0 + 2.0 * a01 * d0 * d1 + a11 * d1 * d1 \
        - 2.0 * lin * c2 + a22 * c2 * c2
    alpha = a22[:, 0]
    mu = -Q / (2.0 * alpha[:, None])
    beta = C0 - Q * Q / (4.0 * alpha[:, None])
    qmin = np.where(mu < 0.0, C0,
                    np.where(mu > VOL - 1.0,
                             C0 + Q * (VOL - 1.0) + alpha[:, None] * (VOL - 1.0) ** 2,
                             beta))
    return C0, Q, alpha, qmin


def plan(x):
    """Input-dependent schedule.

    Returns dict with:
      blocks_of_core [N_CORES, TILES] block ids (slot order),
      seeds_of      {(core, slot): [seed ids]},
      G             [TILES] group count per slot (same for all cores),
    """
    centers, A = _seed_params(x)
    C0, Q, alpha, qmin = _row_coeffs(centers, A)
    brows = _block_rows()                       # [NBLK, 128]
    # surviving seeds per block
    qmin_blk = qmin[:, brows].min(axis=2)       # [S, NBLK]
    alive = qmin_blk <= CUT                     # [S, NBLK]
    counts = alive.sum(axis=0)                  # [NBLK]

    # LPT assignment: 25 blocks per core, balancing total item count
    order = np.argsort(-counts, kind='stable')
    loads = np.zeros(N_CORES, dtype=int)
    nblk = np.zeros(N_CORES, dtype=int)
    blocks_of_core = [[] for _ in range(N_CORES)]
    for b in order:
        elig = [c for c in range(N_CORES) if nblk[c] < TILES]
        c = min(elig, key=lambda c: (loads[c], c))
        blocks_of_core[c].append(int(b))
        loads[c] += int(counts[b])
        nblk[c] += 1
    # slot order: per core, descending item count
    for c in range(N_CORES):
        blocks_of_core[c].sort(key=lambda b: -int(counts[b]))
    # per-slot group counts (max over cores)
    G = []
    for t in range(TILES):
        m = max(int(counts[blocks_of_core[c][t]]) for c in range(N_CORES))
        G.append((m + 2) // 3)
    seeds_of = {}
    for c in range(N_CORES):
        for t in range(TILES):
            b = blocks_of_core[c][t]
            seeds_of[(c, t)] = [int(s) for s in np.nonzero(alive[:, b])[0]]
    return {
        "blocks_of_core": np.array([bc for bc in blocks_of_core]),
        "seeds_of": seeds_of,
        "G": G,
        "centers": centers,
        "A": A,
    }


def _rhs_table():
    """Single k-pattern block [KT, NCOL] shared by every group.

    Columns are (k, s)-interleaved: col = 3*k + s for k in [0,160), seed
    slot s in [0,3) — so the seed fold in the reduce reads contiguously.
    """
    k = np.arange(VOL, dtype=np.float64)
    k2hi, k2lo = _split16(k * k)
    rhs = np.zeros((KT, NCOL), np.float16)
    for sl in range(3):
        r = 7 * sl
        cols = slice(sl, 480, 3)
        rhs[r + 0, cols] = np.float16(1.0)
        rhs[r + 1, cols] = np.float16(1.0)
        rhs[r + 2, cols] = k.astype(np.float16)
        rhs[r + 3, cols] = k.astype(np.float16)
        rhs[r + 4, cols] = k2hi
        rhs[r + 5, cols] = k2lo
        rhs[r + 6, cols] = k2hi
    rhs[0, 480:NCOL] = np.float16(PAD_BIG)  # pad cols via slot0 C0hi row
    return rhs


def _lhs_table(pl, core):
    """[KT, GT*128] fp16 coefficient matrix for one core (GT = sum(G))."""
    centers, A = pl["centers"], pl["A"]
    G = pl["G"]
    GT = sum(G)
    brows = _block_rows()
    C0, Q, alpha, _ = _row_coeffs(centers, A)
    C0hi, C0lo = _split16(C0)
    Qhi, Qlo = _split16(Q)
    Ahi, Alo = _split16(alpha)

    lhs = np.zeros((KT, max(GT, 1) * 128), np.float16)
    if GT == 0:
        lhs[0, :] = np.float16(PAD_BIG)
    gidx = 0
    for t in range(TILES):
        b = pl["blocks_of_core"][core][t]
        rows = brows[b]                        # [128] global row ids
        seeds = pl["seeds_of"][(core, t)]
        for g in range(G[t]):
            base = gidx * 128
            for sl in range(3):
                si = g * 3 + sl
                r = 7 * sl
                if si < len(seeds):
                    s = seeds[si]
                    lhs[r + 0, base:base + 128] = C0hi[s, rows]
                    lhs[r + 1, base:base + 128] = C0lo[s, rows]
                    lhs[r + 2, base:base + 128] = Qhi[s, rows]
                    lhs[r + 3, base:base + 128] = Qlo[s, rows]
                    lhs[r + 4, base:base + 128] = Ahi[s]
                    lhs[r + 5, base:base + 128] = Ahi[s]
                    lhs[r + 6, base:base + 128] = Alo[s]
                else:
                    lhs[r + 0, base:base + 128] = np.float16(PAD_BIG)
            gidx += 1
    return lhs


def _build_nc(G):
    """Build the SPMD program for per-slot group counts G (len TILES)."""
    GT = sum(G)
    nc = bacc.Bacc("TRN2", target_bir_lowering=False, debug=False,
                   num_devices=N_CORES)
    rv = nc.declare_dram_parameter("rv", [TILES * 128, VOL], F32, isOutput=False)
    om = nc.declare_dram_parameter("om", [TILES * 128, VOL], F32, isOutput=False)
    lhs = nc.declare_dram_parameter("lhs", [KT, max(GT, 1) * 128], F16,
                                    isOutput=False)
    rhs = nc.declare_dram_parameter("rhs", [KT, NCOL], F16, isOutput=False)
    partials = nc.declare_dram_parameter("partials", [1, 4], F32, isOutput=True)

    add = mybir.AluOpType.add
    mult = mybir.AluOpType.mult
    Exp = mybir.ActivationFunctionType.Exp
    Sigmoid = mybir.ActivationFunctionType.Sigmoid

    # (tile, local group) stream in program order, packed into PSUM fills of 4
    work = [(t, g) for t in range(TILES) for g in range(G[t])]

    with ExitStack() as ctx:
        tc = ctx.enter_context(tile.TileContext(nc))
        cpool = ctx.enter_context(tc.tile_pool(name="const", bufs=1))
        ppool = ctx.enter_context(tc.tile_pool(name="psum", bufs=2, space="PSUM"))
        gpool = ctx.enter_context(tc.tile_pool(name="g", bufs=6))
        tpool = ctx.enter_context(tc.tile_pool(name="tmp", bufs=4))

        lhs_sb = cpool.tile([KT, max(GT, 1) * 128], F16)
        rhs_sb = cpool.tile([KT, NCOL], F16)
        rv_sb = cpool.tile([128, TILES * VOL], F32)
        om_sb = cpool.tile([128, TILES * VOL], F32)
        rad = cpool.tile([128, TILES * VOL], F32)

        nc.sync.dma_start(rhs_sb[:], rhs[:])
        # split the lhs table DMA so early fills aren't gated on the tail:
        # a small head chunk (first two fills) then three even chunks
        cols = max(GT, 1) * 128
        head = min(8 * 128, cols)
        bounds = [0, head]
        rest = cols - head
        for p in (1, 2, 3):
            bounds.append(head + (rest * p // 3) // 128 * 128 if p < 3 else cols)
        for o, e in zip(bounds[:-1], bounds[1:]):
            if e > o:
                nc.sync.dma_start(lhs_sb[:, o:e], lhs[:, o:e])
        nc.sync.dma_start(
            rv_sb[:].rearrange("p (t k) -> p t k", t=TILES),
            rv[:].rearrange("(t p) k -> p t k", p=128))
        nc.sync.dma_start(
            om_sb[:].rearrange("p (t k) -> p t k", t=TILES),
            om[:].rearrange("(t p) k -> p t k", p=128))

        # tiles with no groups: zero their rad slice
        written = set()
        for t in range(TILES):
            if G[t] == 0:
                nc.vector.memset(rad[:, t * VOL:(t + 1) * VOL], 0.0)
                written.add(t)

        # Phase 1: stream groups through PSUM fills of up to 4.  The
        # seed-sum alternates between VectorE (fused 4D strided reduce)
        # and GpSimd (block adds) to split the DVE bottleneck.
        nfills = (len(work) + 3) // 4
        for fi, f0 in enumerate(range(0, len(work), 4)):
            fill = work[f0:f0 + 4]
            n = len(fill)
            q = ppool.tile([128, 4 * NCOL], F32, tag="q")
            for gl, (t, g) in enumerate(fill):
                gi = f0 + gl   # global group index = lhs slot
                nc.tensor.matmul(
                    q[:, gl * NCOL:(gl + 1) * NCOL],
                    lhsT=lhs_sb[:, gi * 128:(gi + 1) * 128],
                    rhs=rhs_sb[:],
                    start=True, stop=True)
            # Exp skips the 32 pad cols of each 512 block: strided PSUM
            # read, packed 480-wide SBUF write.
            gt = gpool.tile([128, 4 * 480], F32, tag="g")
            q_src = (q[:, :n * NCOL]
                     .rearrange("p (g c) -> p g c", g=n)[:, :, 0:480])
            nc.scalar.activation(
                gt[:, :n * 480].rearrange("p (g c) -> p g c", g=n),
                q_src, Exp, scale=-1.0)
            use_gpsimd = (fi % 5 in (1, 3))
            # accumulate per contiguous same-tile run within the fill
            i0 = 0
            while i0 < n:
                t = fill[i0][0]
                i1 = i0
                while i1 < n and fill[i1][0] == t:
                    i1 += 1
                m = i1 - i0
                rad_t = rad[:, t * VOL:(t + 1) * VOL]
                if not use_gpsimd:
                    # cols of each 480-chunk are (k, s) interleaved
                    red_src = (gt[:, i0 * 480:i1 * 480]
                               .rearrange("p (g k s) -> p k g s", g=m, s=3))
                    if t not in written:
                        nc.vector.tensor_reduce(
                            rad_t, red_src, axis=mybir.AxisListType.XY, op=add)
                        written.add(t)
                    else:
                        tmp = tpool.tile([128, VOL], F32, tag="tmp")
                        nc.vector.tensor_reduce(
                            tmp[:], red_src, axis=mybir.AxisListType.XY, op=add)
                        nc.vector.tensor_add(rad_t, rad_t, tmp[:])
                else:
                    # GpSimd: wide adds over the 480-chunks, then a
                    # strided (k,s) fold
                    eng = nc.gpsimd
                    src = gt[:, i0 * 480:i1 * 480]
                    if m == 1:
                        ssum = src[:, 0:480]
                    elif m == 2:
                        bt = tpool.tile([128, 480], F32, tag="gb")
                        eng.tensor_add(bt[:], src[:, 0:480], src[:, 480:960])
                        ssum = bt[:]
                    elif m == 3:
                        bt = tpool.tile([128, 480], F32, tag="gb")
                        eng.tensor_add(bt[:], src[:, 0:480], src[:, 960:1440])
                        bt2 = tpool.tile([128, 480], F32, tag="gb2")
                        eng.tensor_add(bt2[:], bt[:], src[:, 480:960])
                        ssum = bt2[:]
                    else:  # m == 4
                        bt = tpool.tile([128, 960], F32, tag="gbw")
                        eng.tensor_add(bt[:], src[:, 0:960], src[:, 960:1920])
                        bt2 = tpool.tile([128, 480], F32, tag="gb2")
                        eng.tensor_add(bt2[:], bt[:, 0:480], bt[:, 480:960])
                        ssum = bt2[:]
                    ks = ssum.rearrange("p (k s) -> p k s", s=3)
                    u = tpool.tile([128, VOL], F32, tag="gu")
                    eng.tensor_add(u[:], ks[:, :, 0], ks[:, :, 1])
                    if t not in written:
                        eng.tensor_add(rad_t, u[:], ks[:, :, 2])
                        written.add(t)
                    else:
                        u2 = tpool.tile([128, VOL], F32, tag="gu2")
                        eng.tensor_add(u2[:], u[:], ks[:, :, 2])
                        eng.tensor_add(rad_t, rad_t, u2[:])
                i0 = i1

        # Phase 2: masked sums
        CH = 2
        CW = TILES * VOL // CH
        eparts = cpool.tile([128, CH], F32)
        tparts = cpool.tile([128, CH], F32)
        euparts = cpool.tile([128, CH], F32)
        uparts = cpool.tile([128, CH], F32)
        acc4 = cpool.tile([128, 4], F32)
        bneg1 = cpool.tile([128, 1], F32)
        bneg50 = cpool.tile([128, 1], F32)
        nc.vector.memset(bneg1[:], -1.0)
        nc.vector.memset(bneg50[:], -50.0)
        for c in range(CH):
            sl = slice(c * CW, (c + 1) * CW)
            eff = tpool.tile([128, CW], F32, tag="eff")
            nc.vector.scalar_tensor_tensor(
                eff[:], rad[:, sl], 1.0, rv_sb[:, sl], mult, mult,
                accum_out=eparts[:, c:c + 1])
            nc.scalar.activation(eff[:], eff[:], Sigmoid, bias=bneg1[:],
                                 scale=1.0, accum_out=euparts[:, c:c + 1])
            outv = tpool.tile([128, CW], F32, tag="outv")
            nc.vector.tensor_mul(outv[:], rad[:, sl], om_sb[:, sl])
            nc.scalar.activation(outv[:], outv[:], Sigmoid, bias=bneg50[:],
                                 scale=100.0, accum_out=uparts[:, c:c + 1])
            nc.vector.tensor_reduce(tparts[:, c:c + 1], rad[:, sl],
                                    axis=mybir.AxisListType.X, op=add)
        for idx, p4 in enumerate([eparts, tparts, euparts, uparts]):
            nc.vector.tensor_reduce(acc4[:, idx:idx + 1], p4[:],
                                    axis=mybir.AxisListType.X, op=add)
        ones = cpool.tile([128, 1], F32)
        nc.vector.memset(ones[:], 1.0)
        accq = ppool.tile([1, 4], F32, tag="q")
        nc.tensor.matmul(accq[:], lhsT=ones[:], rhs=acc4[:], start=True,
                         stop=True)
        res = tpool.tile([1, 4], F32, tag="res")
        nc.scalar.copy(res[:], accq[:])
        nc.sync.dma_start(partials[:], res[:])
    nc.compile()
    return nc


_NC_CACHE = {}
LAST_RESULT = None


def _get_nc(G):
    key = tuple(G)
    if key not in _NC_CACHE:
        _NC_CACHE[key] = _build_nc(list(G))
    return _NC_CACHE[key]


def kernel(x, radiation_volume, outside_mask):
    from concourse.bass_utils import run_bass_kernel_spmd

    pl = plan(x)
    nc = _get_nc(pl["G"])
    rv2 = np.asarray(radiation_volume, np.float32).reshape(VOL * VOL, VOL)
    om2 = np.asarray(outside_mask, np.float32).reshape(VOL * VOL, VOL)
    brows = _block_rows()
    rhs = _rhs_table()
    in_maps = []
    for c in range(N_CORES):
        rows = brows[pl["blocks_of_core"][c]].reshape(-1)   # [3200]
        in_maps.append({
            "rv": np.ascontiguousarray(rv2[rows]),
            "om": np.ascontiguousarray(om2[rows]),
            "lhs": np.ascontiguousarray(_lhs_table(pl, c)),
            "rhs": rhs,
        })
    out = run_bass_kernel_spmd(nc, in_maps, list(range(N_CORES)))
    global LAST_RESULT
    LAST_RESULT = out
    parts = np.stack([out.results[i]["partials"][0] for i in range(N_CORES)])
    E, T, EU, U = parts.sum(axis=0, dtype=np.float64)
    num_target = float(np.sum(rv2, dtype=np.float64))
    loss = (DVH_RATE - EU / num_target) + (1.0 - E / T) + U / num_target
    return np.array(loss, dtype=np.float32)
